# revision 1
# baseline (speedup 1.0000x reference)
"""Trainium2 Bass kernel for a small autoregressive transformer block with
local-windowed causal attention and a large (16k) vocab head.

Data-parallel over batch: batch item b runs on NeuronCore b (8 cores).
Per core:
  h   = embed_tab[x] + pos                      [1024, 512]
  q/k/v = h @ Wq/k/v (+b)                       [1024, 512]
  s   = q @ k^T / sqrt(D) + local_causal_mask   (banded, window <= 298)
  o   = softmax(s) @ v @ Wo (+bo)
  h1  = LN(h + o);  f = relu(h1@W1+b1)@W2+b2;  h2 = LN(h1 + f)
  out = h2 @ Wh (+bh)                           [1024, 16384]

All matmuls run as float32r (full-rate fp32 with N=512 moving dim).
kernel(**inputs) takes full unsharded inputs, returns [8, 1024, 16384] f32.
"""

import math
import numpy as np

import concourse.bass as bass
import concourse.mybir as mybir
import concourse.tile as tile
from concourse import bacc
from concourse.bass_utils import run_bass_kernel_spmd
from concourse.masks import make_identity

# ---- problem constants (hardcoded per contract) ----
GH = 32
GW = 32
SEQ = 1024
WIN = 9
D = 512
DFF = 1024
VOCAB = 16384
EPS = 1e-5
NEG = -1e30

P = 128
NT = SEQ // P        # 8 token chunks
DC = D // P          # 4 d chunks
FC = DFF // P        # 8 dff chunks
NV = VOCAB // 512    # 32 vocab chunks
INV_SQRT_D = 1.0 / math.sqrt(D)

F32 = mybir.dt.float32
F32R = mybir.dt.float32r
BF16 = mybir.dt.bfloat16
I32 = mybir.dt.int32
HEAD_BF16 = True
OUT_BF16 = True
AF = mybir.ActivationFunctionType


def _window_start(i: int) -> int:
    # k-window [ws, ws+512) covers all allowed keys for query chunk i
    # (max lookback is WIN*GW + WIN = 297 < 384).
    return 128 * max(0, i - 3)


def _mask_tiles() -> np.ndarray:
    idx = np.arange(SEQ)
    r, c = idx // GW, idx % GW
    allow = (
        (np.abs(r[:, None] - r[None, :]) <= WIN)
        & (np.abs(c[:, None] - c[None, :]) <= WIN)
        & (idx[None, :] <= idx[:, None])
    )
    maskf = np.where(allow, 0.0, NEG).astype(np.float32)
    tiles = np.empty((NT, P, 512), np.float32)
    for i in range(NT):
        ws = _window_start(i)
        tiles[i] = maskf[i * P : (i + 1) * P, ws : ws + 512]
    return tiles


def _r(ap):
    """bitcast to float32r for full-rate fp32 matmul."""
    return ap.bitcast(F32R)


def _bcast_ap(a: bass.AP) -> bass.AP:
    """[n] DRAM vector AP -> [P, n] partition-broadcast DMA source."""
    return bass.AP(tensor=a.tensor, offset=a.offset, ap=[[0, P], *a.ap])


def _build_program(flags: dict, wh_bufs: int = 6, msk_bufs: int = 6, lean: bool = False) -> bass.Bass:
    nc = bacc.Bacc("TRN2", target_bir_lowering=False)

    # ---------- I/O ----------
    x_d = nc.declare_dram_parameter("x", [SEQ], I32, False)
    emb_d = nc.declare_dram_parameter("emb", [VOCAB, D], F32, False)
    pos_d = nc.declare_dram_parameter("pos", [SEQ, D], F32, False)
    msk_d = nc.declare_dram_parameter("maskt", [NT, P, 512], F32, False)
    wq_d = nc.declare_dram_parameter("wq", [D, D], F32, False)
    wk_d = nc.declare_dram_parameter("wk", [D, D], F32, False)
    wv_d = nc.declare_dram_parameter("wv", [D, D], F32, False)
    wo_d = nc.declare_dram_parameter("wo", [D, D], F32, False)
    w1_d = nc.declare_dram_parameter("w1", [D, DFF], F32, False)
    w2_d = nc.declare_dram_parameter("w2", [DFF, D], F32, False)
    wh_d = nc.declare_dram_parameter("wh", [D, VOCAB], BF16 if HEAD_BF16 else F32, False)
    dp = lambda name, shape: nc.declare_dram_parameter(name, shape, F32, False)
    bq_d = dp("bq", [D]) if flags["bq"] else None
    bk_d = dp("bk", [D]) if flags["bk"] else None
    bv_d = dp("bv", [D]) if flags["bv"] else None
    bo_d = dp("bo", [D]) if flags["bo"] else None
    b1_d = dp("b1", [DFF]) if flags["b1"] else None
    b2_d = dp("b2", [D]) if flags["b2"] else None
    bh_d = dp("bh", [VOCAB]) if flags["bh"] else None
    g1_d = dp("g1", [D]) if flags["g1"] else None
    be1_d = dp("be1", [D]) if flags["be1"] else None
    g2_d = dp("g2", [D]) if flags["g2"] else None
    be2_d = dp("be2", [D]) if flags["be2"] else None
    out_d = nc.declare_dram_parameter("out", [SEQ, VOCAB], BF16 if OUT_BF16 else F32, True)

    with tile.TileContext(nc) as tc:
        # ----- whole-kernel pools -----
        const = tc.alloc_tile_pool(name="const", bufs=1)
        small = tc.alloc_tile_pool(name="small", bufs=8)
        psum = tc.alloc_tile_pool(name="psA", bufs=6, space="PSUM")
        psum_t = tc.alloc_tile_pool(name="psT", bufs=2, space="PSUM")
        opool = tc.alloc_tile_pool(name="outev", bufs=2, side="right")
        p_h2T = tc.alloc_tile_pool(name="h2Tp", bufs=1, side="right")

        ident_f = const.tile([P, P], F32, tag="ident_f")
        ident = const.tile([P, P], F32R, tag="ident")
        eps_t = const.tile([P, 1], F32, tag="eps")
        nc.vector.memset(eps_t[:], EPS)
        x_sb = const.tile([P, NT], I32, tag="x_sb")
        nc.sync.dma_start(out=x_sb[:], in_=x_d[:].rearrange("(j p) -> p j", p=P))

        def load_col_bias(handle, nchunks, tag):
            # [nchunks*P] DRAM -> [P, nchunks] (chunk m in column m)
            t = const.tile([P, nchunks], F32, tag=tag)
            nc.sync.dma_start(out=t[:], in_=handle[:].rearrange("(m p) -> p m", p=P))
            return t

        def load_bcast(handle, n, tag):
            t = const.tile([P, n], F32, tag=tag)
            nc.sync.dma_start(out=t[:], in_=_bcast_ap(handle[:]))
            return t

        bq_sb = load_col_bias(bq_d, DC, "bq") if bq_d else None
        bk_sb = load_col_bias(bk_d, DC, "bk") if bk_d else None
        b1_sb = load_col_bias(b1_d, FC, "b1") if b1_d else None
        bv_bc = load_bcast(bv_d, D, "bv") if bv_d else None
        bo_bc = load_bcast(bo_d, D, "bo") if bo_d else None
        b2_bc = load_bcast(b2_d, D, "b2") if b2_d else None
        g1_bc = load_bcast(g1_d, D, "g1") if g1_d else None
        be1_bc = load_bcast(be1_d, D, "be1") if be1_d else None
        g2_bc = load_bcast(g2_d, D, "g2") if g2_d else None
        be2_bc = load_bcast(be2_d, D, "be2") if be2_d else None

        h2T = [p_h2T.tile([P, DC, P], BF16 if HEAD_BF16 else F32R, tag=f"h2T{j}", name=f"h2T{j}") for j in range(NT)]

        # ----- phase A pools (left, LIFO) -----
        p_woh = tc.alloc_tile_pool(name="woh", bufs=1)         # wo, h  (-> stage 4)
        wo_sb = p_woh.tile([P, DC, D], F32R, tag="wo")
        h_sb = p_woh.tile([P, NT, D], F32R, tag="h")

        p_oT = tc.alloc_tile_pool(name="oTp", bufs=1)          # oT    (-> stage 4)
        oT = p_oT.tile([P, DC, SEQ], F32R, tag="oT")

        p_v = tc.alloc_tile_pool(name="vp", bufs=1)            # v (-> wave 2)
        v_sb = p_v.tile([P, NT, D], F32R, tag="v")
        p_at = tc.alloc_tile_pool(name="attnw", bufs=3)        # softmax work (-> stage 4)
        p_qk = tc.alloc_tile_pool(name="qkp", bufs=1)          # qT,kT (-> wave 1)
        qT = p_qk.tile([P, DC, SEQ], F32R, tag="qT")
        kT = p_qk.tile([P, DC, SEQ], F32R, tag="kT")

        p_wq = tc.alloc_tile_pool(name="wqp", bufs=1)          # wq,wk,wv,hT (-> stage 2)
        wq_sb = p_wq.tile([P, DC, D], F32R, tag="wq")
        wk_sb = p_wq.tile([P, DC, D], F32R, tag="wk")
        wv_sb = p_wq.tile([P, DC, D], F32R, tag="wv")
        hT = p_wq.tile([P, DC, SEQ], F32R, tag="hT")

        # ---------- stage 1: embedding gather + positional + transpose ----------
        for jj in range(NT):
            nc.gpsimd.indirect_dma_start(
                out=h_sb[:, jj, :],
                out_offset=None,
                in_=_r(emb_d[:]),
                in_offset=bass.IndirectOffsetOnAxis(ap=x_sb[:, jj : jj + 1], axis=0),
            )

        make_identity(nc, ident_f[:])
        nc.vector.tensor_copy(out=ident[:], in_=ident_f[:])

        def s1_add(j):
            pos_t = p_wq.tile([P, D], F32, tag="pos", bufs=3, name=f"pos{j}")
            nc.sync.dma_start(out=pos_t[:], in_=pos_d[j * P : (j + 1) * P, :])
            nc.vector.tensor_add(out=h_sb[:, j, :], in0=h_sb[:, j, :], in1=pos_t[:])

        def s1_trans(j):
            pt = psum_t.tile([P, 512], F32, tag="pt", name=f"s1pt{j}")
            for m in range(DC):
                nc.tensor.transpose(
                    out=_r(pt[:, m * P : (m + 1) * P]),
                    in_=_r(h_sb[:, j, m * P : (m + 1) * P]),
                    identity=_r(ident[:]),
                )
            nc.scalar.copy(out=hT[:, :, j * P : (j + 1) * P], in_=pt[:])

        for k in range(NT + 1):
            if k < NT:
                s1_add(k)
            if k >= 1:
                s1_trans(k - 1)

        # weight DMAs issued after stage-1 loads so embeddings/pos win the queue
        for w_sb, w_d in ((wq_sb, wq_d), (wk_sb, wk_d), (wv_sb, wv_d), (wo_sb, wo_d)):
            nc.sync.dma_start(out=w_sb[:], in_=_r(w_d[:].rearrange("(k p) o -> p k o", p=P)))

        # ---------- stage 2: qT / kT (d-major), v (token-major) ----------
        # t-major order: all groups needing hT[0:512] first (PE is in-order)
        for t in range(SEQ // 512):
            for (wt, bt, dst) in ((wq_sb, bq_sb, qT), (wk_sb, bk_sb, kT)):
                for m in range(DC):
                    ps = psum.tile([P, 512], F32, tag="ps")
                    for ki in range(DC):
                        nc.tensor.matmul(
                            ps[:],
                            _r(wt[:, ki, m * P : (m + 1) * P]),
                            _r(hT[:, ki, t * 512 : (t + 1) * 512]),
                            start=(ki == 0),
                            stop=(ki == DC - 1),
                        )
                    dslc = dst[:, m, t * 512 : (t + 1) * 512]
                    if bt is not None:
                        nc.scalar.activation(
                            out=dslc, in_=ps[:], func=AF.Identity,
                            bias=bt[:, m : m + 1], scale=1.0,
                        )
                    elif dst is kT:
                        nc.vector.tensor_copy(out=dslc, in_=ps[:])
                    else:
                        nc.scalar.copy(out=dslc, in_=ps[:])
            for j in range(4 * t, 4 * t + 4):
                ps = psum.tile([P, 512], F32, tag="ps")
                for ki in range(DC):
                    nc.tensor.matmul(
                        ps[:],
                        _r(hT[:, ki, j * P : (j + 1) * P]),
                        _r(wv_sb[:, ki, :]),
                        start=(ki == 0),
                        stop=(ki == DC - 1),
                    )
                if bv_bc is not None:
                    nc.vector.tensor_add(out=v_sb[:, j, :], in0=ps[:], in1=bv_bc[:])
                else:
                    nc.vector.tensor_copy(out=v_sb[:, j, :], in_=ps[:])

        p_wq.release()

        # ---------- stage 3 wave 1: scores + softmax ----------
        attns = []
        recips = []
        for i in range(NT):
            ws = _window_start(i)
            nw = min(512, max(256, (i + 1) * P))  # live window (>=256 keeps f32r fast)
            ps_s = psum.tile([P, 512], F32, tag="ps")
            for ki in range(DC):
                nc.tensor.matmul(
                    ps_s[:, :nw],
                    _r(qT[:, ki, i * P : (i + 1) * P]),
                    _r(kT[:, ki, ws : ws + nw]),
                    start=(ki == 0),
                    stop=(ki == DC - 1),
                )
            msk_t = p_at.tile([P, 512], F32, tag="msk", bufs=msk_bufs)
            nc.sync.dma_start(out=msk_t[:], in_=msk_d[i])
            s_t = p_at.tile([P, 512], F32, tag="s_t", bufs=3)
            nc.vector.tensor_add(out=s_t[:, :nw], in0=ps_s[:, :nw], in1=msk_t[:, :nw])
            attn = p_at.tile([P, 512], F32R, tag="attn", bufs=NT, name=f"attn{i}")
            denom = small.tile([P, 1], F32, tag="denom")
            nc.scalar.activation(
                out=attn[:, :nw], in_=s_t[:, :nw], func=AF.Exp,
                bias=0.0, scale=INV_SQRT_D,
                accum_out=denom[:, 0:1],
            )
            recip = small.tile([P, 1], F32, tag="recip", bufs=NT, name=f"recip{i}")
            nc.vector.reciprocal(out=recip[:], in_=denom[:])
            attns.append(attn)
            recips.append(recip)

        p_qk.release()

        # ----- right-side pools for FFN phase -----
        whpool = tc.alloc_tile_pool(name="whstream", bufs=wh_bufs, side="right")
        p_h1 = tc.alloc_tile_pool(name="h1p", bufs=1, side="right")
        h1_sb = p_h1.tile([P, NT, D], F32R, tag="h1")
        h1T = p_h1.tile([P, DC, SEQ], F32R, tag="h1T")
        w1_sb = p_h1.tile([P, DC, DFF], F32R, tag="w1")
        nc.sync.dma_start(out=w1_sb[:], in_=_r(w1_d[:].rearrange("(k p) o -> p k o", p=P)))

        # ---------- stage 3 wave 2 + stage 4, software-pipelined ----------
        p_st4 = tc.alloc_tile_pool(name="st4", bufs=3)
        attnTs = [None] * NT
        o_ts = [None] * NT

        def w2_a(i):  # attn transposes + attnT eviction
            ws = _window_start(i)
            kb0 = ws // P
            nkb = min(DC, i - kb0 + 1)
            pt = psum_t.tile([P, 512], F32, tag="pt", name=f"atp{i}")
            for kk in range(nkb):
                nc.tensor.transpose(
                    out=_r(pt[:, kk * P : (kk + 1) * P]),
                    in_=_r(attns[i][:, kk * P : (kk + 1) * P]),
                    identity=_r(ident[:]),
                )
            attnT = p_at.tile([P, 512], F32R, tag="attnT", bufs=3, name=f"attnT{i}")
            nc.scalar.copy(out=attnT[:, : nkb * P], in_=pt[:, : nkb * P])
            attnTs[i] = attnT

        def w2_b(i):  # o matmuls + scale
            ws = _window_start(i)
            kb0 = ws // P
            nkb = min(DC, i - kb0 + 1)
            ps_o = psum.tile([P, 512], F32, tag="ps", name=f"pso{i}")
            for kk in range(nkb):
                nc.tensor.matmul(
                    ps_o[:],
                    attnTs[i][:, kk * P : (kk + 1) * P],
                    _r(v_sb[:, kb0 + kk, :]),
                    start=(kk == 0),
                    stop=(kk == nkb - 1),
                )
            o_t = p_at.tile([P, D], F32R, tag="o_t", bufs=3, name=f"o_t{i}")
            nc.vector.tensor_scalar_mul(out=o_t[:], in0=ps_o[:], scalar1=recips[i][:, 0:1])
            o_ts[i] = o_t

        def w2_c(i):  # oT transposes + eviction
            pt2 = psum_t.tile([P, 512], F32, tag="pt", name=f"otp{i}")
            for m in range(DC):
                nc.tensor.transpose(
                    out=_r(pt2[:, m * P : (m + 1) * P]),
                    in_=_r(o_ts[i][:, m * P : (m + 1) * P]),
                    identity=_r(ident[:]),
                )
            nc.vector.tensor_copy(out=oT[:, :, i * P : (i + 1) * P], in_=pt2[:])

        def s4_proj(j):  # attn projection + residual + LN1 (no transpose)
            ps = psum.tile([P, 512], F32, tag="ps", name=f"psp{j}")
            for m in range(DC):
                nc.tensor.matmul(
                    ps[:],
                    _r(oT[:, m, j * P : (j + 1) * P]),
                    _r(wo_sb[:, m, :]),
                    start=(m == 0),
                    stop=(m == DC - 1),
                )
            r1 = p_st4.tile([P, D], F32, tag="r1", name=f"r1_{j}")
            nc.vector.tensor_add(out=r1[:], in0=h_sb[:, j, :], in1=ps[:])
            if bo_bc is not None:
                nc.vector.tensor_add(out=r1[:], in0=r1[:], in1=bo_bc[:])
            stats = small.tile([P, 6], F32, tag="stats")
            nc.vector.bn_stats(out=stats[:], in_=r1[:])
            mv = small.tile([P, 2], F32, tag="mv")
            nc.vector.bn_aggr(out=mv[:], in_=stats[:])
            stdt = small.tile([P, 1], F32, tag="stdt")
            nc.scalar.activation(
                out=stdt[:], in_=mv[:, 1:2], func=AF.Sqrt,
                bias=eps_t[:, 0:1], scale=1.0,
            )
            rstd = small.tile([P, 1], F32, tag="rstd")
            nc.vector.reciprocal(out=rstd[:], in_=stdt[:])
            nc.vector.tensor_scalar(
                out=h1_sb[:, j, :], in0=r1[:],
                scalar1=mv[:, 0:1], scalar2=rstd[:, 0:1],
                op0=mybir.AluOpType.subtract, op1=mybir.AluOpType.mult,
            )
            if g1_bc is not None:
                nc.vector.tensor_mul(out=h1_sb[:, j, :], in0=h1_sb[:, j, :], in1=g1_bc[:])
            if be1_bc is not None:
                nc.vector.tensor_add(out=h1_sb[:, j, :], in0=h1_sb[:, j, :], in1=be1_bc[:])

        def s4_trans(j):  # h1 transposes + h1T eviction
            pt3 = psum_t.tile([P, 512], F32, tag="pt", name=f"h1p{j}")
            for m in range(DC):
                nc.tensor.transpose(
                    out=_r(pt3[:, m * P : (m + 1) * P]),
                    in_=_r(h1_sb[:, j, m * P : (m + 1) * P]),
                    identity=_r(ident[:]),
                )
            nc.scalar.copy(out=h1T[:, :, j * P : (j + 1) * P], in_=pt3[:])

        for k in range(NT + 4):
            if k < NT:
                w2_a(k)
            if 1 <= k < NT + 1:
                w2_b(k - 1)
            if 2 <= k < NT + 2:
                w2_c(k - 2)
            if 3 <= k < NT + 3:
                s4_proj(k - 3)
            if 4 <= k:
                s4_trans(k - 4)

        p_st4.release()
        p_at.release()
        p_v.release()
        p_oT.release()
        p_woh.release()

        p_w12 = tc.alloc_tile_pool(name="w12", bufs=1, side="right")
        w2_sb = p_w12.tile([P, FC, D], F32R, tag="w2")
        nc.sync.dma_start(out=w2_sb[:], in_=_r(w2_d[:].rearrange("(k p) o -> p k o", p=P)))

        # ---------- stage 5: FFN up, f1T = relu(W1^T @ h1T + b1) ----------
        p_f1 = tc.alloc_tile_pool(name="f1p", bufs=1, side="right")
        f1T = p_f1.tile([P, FC, SEQ], F32R, tag="f1T")
        def ffn1_group(n, t):
            ps = psum.tile([P, 512], F32, tag="ps", name=f"psf{n}_{t}")
            for ki in range(DC):
                nc.tensor.matmul(
                    ps[:],
                    _r(w1_sb[:, ki, n * P : (n + 1) * P]),
                    _r(h1T[:, ki, t * 512 : (t + 1) * 512]),
                    start=(ki == 0),
                    stop=(ki == DC - 1),
                )
            fslc = f1T[:, n, t * 512 : (t + 1) * 512]
            if b1_sb is not None:
                nc.vector.tensor_scalar(
                    out=fslc, in0=ps[:],
                    scalar1=b1_sb[:, n : n + 1], scalar2=0.0,
                    op0=mybir.AluOpType.add, op1=mybir.AluOpType.max,
                )
            else:
                nc.vector.tensor_scalar_max(out=fslc, in0=ps[:], scalar1=0.0)

        # ---------- stage 6: FFN down + residual + LN2 (pipelined) ----------
        def s6_main(j):
            ps = psum.tile([P, 512], F32, tag="ps", name=f"ps6_{j}")
            for n in range(FC):
                nc.tensor.matmul(
                    ps[:],
                    _r(f1T[:, n, j * P : (j + 1) * P]),
                    _r(w2_sb[:, n, :]),
                    start=(n == 0),
                    stop=(n == FC - 1),
                )
            r2 = p_f1.tile([P, D], F32, tag="r2", bufs=3, name=f"r2_{j}")
            nc.vector.tensor_add(out=r2[:], in0=h1_sb[:, j, :], in1=ps[:])
            if b2_bc is not None:
                nc.vector.tensor_add(out=r2[:], in0=r2[:], in1=b2_bc[:])
            stats = small.tile([P, 6], F32, tag="stats")
            nc.vector.bn_stats(out=stats[:], in_=r2[:])
            mv = small.tile([P, 2], F32, tag="mv")
            nc.vector.bn_aggr(out=mv[:], in_=stats[:])
            stdt = small.tile([P, 1], F32, tag="stdt")
            nc.scalar.activation(
                out=stdt[:], in_=mv[:, 1:2], func=AF.Sqrt,
                bias=eps_t[:, 0:1], scale=1.0,
            )
            rstd = small.tile([P, 1], F32, tag="rstd")
            nc.vector.reciprocal(out=rstd[:], in_=stdt[:])
            h2_t = p_f1.tile([P, D], F32R, tag="h2_t", bufs=3, name=f"h2t_{j}")
            nc.vector.tensor_scalar(
                out=h2_t[:], in0=r2[:],
                scalar1=mv[:, 0:1], scalar2=rstd[:, 0:1],
                op0=mybir.AluOpType.subtract, op1=mybir.AluOpType.mult,
            )
            if g2_bc is not None:
                nc.vector.tensor_mul(out=h2_t[:], in0=h2_t[:], in1=g2_bc[:])
            if be2_bc is not None:
                nc.vector.tensor_add(out=h2_t[:], in0=h2_t[:], in1=be2_bc[:])
            return h2_t

        h2ts = [None] * NT

        def s6_trans(j):
            pt = psum_t.tile([P, 512], F32, tag="pt", name=f"h2p{j}")
            for m in range(DC):
                nc.tensor.transpose(
                    out=_r(pt[:, m * P : (m + 1) * P]),
                    in_=_r(h2ts[j][:, m * P : (m + 1) * P]),
                    identity=_r(ident[:]),
                )
            nc.scalar.copy(out=h2T[j][:, :, :], in_=pt[:])

        # head chunks for vc=0,1 interleaved into stage-6 so PE fills LN waits
        whv0 = whpool.tile([P, DC, 512], BF16 if HEAD_BF16 else F32R, tag="whv", name="whv0")
        nc.sync.dma_start(out=whv0[:], in_=wh_d[:].rearrange("(k p) v -> p k v", p=P)[:, :, 0:512])
        otile0 = opool.tile([P, NT, 512], BF16 if OUT_BF16 else F32, tag="ot", name="otile0")
        whv1 = whpool.tile([P, DC, 512], BF16 if HEAD_BF16 else F32R, tag="whv", name="whv1")
        nc.sync.dma_start(out=whv1[:], in_=wh_d[:].rearrange("(k p) v -> p k v", p=P)[:, :, 512:1024])
        otile1 = opool.tile([P, NT, 512], BF16 if OUT_BF16 else F32, tag="ot", name="otile1")

        def head_j(whv, otile, j, toggle):
            ps = psum.tile([P, 512], F32, tag="ps", name=f"psh{toggle}_{j}")
            for ki in range(DC):
                nc.tensor.matmul(
                    ps[:],
                    h2T[j][:, ki, :],
                    whv[:, ki, :],
                    start=(ki == 0),
                    stop=(ki == DC - 1),
                )
            if bh_sb_for(toggle) is not None:
                nc.vector.tensor_add(out=otile[:, j, :], in0=ps[:], in1=bh_sb_for(toggle)[:])
            elif j % 2 == 0:
                nc.vector.tensor_copy(out=otile[:, j, :], in_=ps[:])
            else:
                nc.scalar.copy(out=otile[:, j, :], in_=ps[:])

        _bh_tiles = {}

        def bh_sb_for(key):
            return _bh_tiles.get(key)

        if bh_d is not None:
            bh0 = whpool.tile([P, 512], F32, tag="bh", bufs=2, name="bh0")
            nc.sync.dma_start(out=bh0[:], in_=_bcast_ap(bh_d[0:512]))
            _bh_tiles[0] = bh0
            bh1 = whpool.tile([P, 512], F32, tag="bh", bufs=2, name="bh1")
            nc.sync.dma_start(out=bh1[:], in_=_bcast_ap(bh_d[512:1024]))
            _bh_tiles[1] = bh1

        for t in range(SEQ // 512):
            for n in range(FC):
                ffn1_group(n, t)
                if t == 1 and n % 2 == 1:
                    j = n // 2
                    h2ts[j] = s6_main(j)

        for k in range(NT + 3):
            if 4 <= k < NT:
                h2ts[k] = s6_main(k)
            if 1 <= k <= NT:
                s6_trans(k - 1)
            if 2 <= k <= NT + 1:
                head_j(whv0, otile0, k - 2, 0)
            if 3 <= k <= NT + 2:
                head_j(whv1, otile1, k - 3, 1)
        out_rr = out_d[:].rearrange("(j p) v -> p j v", p=P)
        nc.sync.dma_start(out=out_rr[:, :, 0:512], in_=otile0[:])
        nc.sync.dma_start(out=out_rr[:, :, 512:1024], in_=otile1[:])

        p_f1.release()
        p_w12.release()
        p_h1.release()

        # ---------- stage 7: vocab head (vc >= 1) ----------
        wh_r = wh_d[:].rearrange("(k p) v -> p k v", p=P)
        out_r = out_d[:].rearrange("(j p) v -> p j v", p=P)
        for vc in range(2, NV):
            whv = whpool.tile([P, DC, 512], BF16 if HEAD_BF16 else F32R, tag="whv")
            nc.sync.dma_start(out=whv[:], in_=wh_r[:, :, vc * 512 : (vc + 1) * 512])
            if bh_d is not None:
                bh_bc = whpool.tile([P, 512], F32, tag="bh", bufs=2, name=f"bh{vc}")
                nc.sync.dma_start(
                    out=bh_bc[:], in_=_bcast_ap(bh_d[vc * 512 : (vc + 1) * 512])
                )
                _bh_tiles[vc] = bh_bc
            otile = opool.tile([P, NT, 512], BF16 if OUT_BF16 else F32, tag="ot")
            for j in range(NT):
                head_j(whv, otile, j, vc)
            nc.sync.dma_start(
                out=out_r[:, :, vc * 512 : (vc + 1) * 512], in_=otile[:]
            )

        whpool.release()
        p_h2T.release()
        opool.release()
        psum_t.release()
        psum.release()
        small.release()
        const.release()

    nc.finalize()
    return nc


_PROGRAM_CACHE: dict = {}


def _get_program(flags: dict) -> bass.Bass:
    key = tuple(sorted(flags.items()))
    if key not in _PROGRAM_CACHE:
        _PROGRAM_CACHE[key] = _build_program(flags)
    return _PROGRAM_CACHE[key]


def _prep(x, embed_tab, row_embed, col_embed, Wq, bq, Wk, bk, Wv, bv, Wo, bo,
          ln1_g, ln1_b, W1, b1, W2, b2, ln2_g, ln2_b, Wh, bh):
    """Shared host-side prep: flags, common input map, per-core x shards."""
    f32c = lambda a: np.ascontiguousarray(np.asarray(a, dtype=np.float32))
    x = np.asarray(x)
    B = x.shape[0]
    assert x.shape == (B, SEQ)

    arrs = dict(
        emb=f32c(embed_tab), wq=f32c(Wq), wk=f32c(Wk), wv=f32c(Wv), wo=f32c(Wo),
        w1=f32c(W1), w2=f32c(W2),
    )
    if HEAD_BF16:
        import ml_dtypes
        arrs["wh"] = np.ascontiguousarray(
            np.asarray(Wh, dtype=np.float32).astype(ml_dtypes.bfloat16)
        )
    else:
        arrs["wh"] = f32c(Wh)
    pos = np.concatenate(
        [np.repeat(f32c(row_embed), GW, axis=0), np.tile(f32c(col_embed), (GH, 1))],
        axis=-1,
    )
    arrs["pos"] = np.ascontiguousarray(pos, dtype=np.float32)
    arrs["maskt"] = _mask_tiles()

    bias_map = dict(
        bq=f32c(bq), bk=f32c(bk), bv=f32c(bv), bo=f32c(bo), b1=f32c(b1),
        b2=f32c(b2), bh=f32c(bh), be1=f32c(ln1_b), be2=f32c(ln2_b),
    )
    gain_map = dict(g1=f32c(ln1_g), g2=f32c(ln2_g))
    flags = {k: bool(np.any(v)) for k, v in bias_map.items()}
    flags.update({k: bool(np.any(v != 1.0)) for k, v in gain_map.items()})
    for k, v in {**bias_map, **gain_map}.items():
        if flags[k]:
            arrs[k] = v

    xs = [np.ascontiguousarray(x[c].astype(np.int32)) for c in range(B)]
    return flags, arrs, xs, B


def kernel(**inputs):
    flags, arrs, xs, B = _prep(**inputs)
    nc = _get_program(flags)
    core_ids = list(range(8))
    in_maps = [{**arrs, "x": xs[c % B]} for c in core_ids]
    res = run_bass_kernel_spmd(nc, in_maps, core_ids)
    out = np.stack([res.results[c]["out"] for c in range(B)], axis=0)
    return np.asarray(out, dtype=np.float32)



# revision 16
# speedup vs baseline: 1.1812x; 1.1812x over previous
"""Trainium2 Bass kernel for a small autoregressive transformer block with
local-windowed causal attention and a large (16k) vocab head.

Data-parallel over batch: batch item b runs on NeuronCore b (8 cores).
Per core:
  h   = embed_tab[x] + pos                      [1024, 512]
  q/k/v = h @ Wq/k/v (+b)                       [1024, 512]
  s   = q @ k^T / sqrt(D) + local_causal_mask   (banded, window <= 298)
  o   = softmax(s) @ v @ Wo (+bo)
  h1  = LN(h + o);  f = relu(h1@W1+b1)@W2+b2;  h2 = LN(h1 + f)
  out = h2 @ Wh (+bh)                           [1024, 16384]

All matmuls run as float32r (full-rate fp32 with N=512 moving dim).
kernel(**inputs) takes full unsharded inputs, returns [8, 1024, 16384] f32.
"""

import math
import numpy as np

import concourse.bass as bass
import concourse.mybir as mybir
import concourse.tile as tile
from concourse import bacc
from concourse.bass_utils import run_bass_kernel_spmd
from concourse.masks import make_identity

# ---- problem constants (hardcoded per contract) ----
GH = 32
GW = 32
SEQ = 1024
WIN = 9
D = 512
DFF = 1024
VOCAB = 16384
EPS = 1e-5
NEG = -1e30

P = 128
NT = SEQ // P        # 8 token chunks
DC = D // P          # 4 d chunks
FC = DFF // P        # 8 dff chunks
NV = VOCAB // 512    # 32 vocab chunks
INV_SQRT_D = 1.0 / math.sqrt(D)

F32 = mybir.dt.float32
F32R = mybir.dt.float32r
BF16 = mybir.dt.bfloat16
F8 = mybir.dt.float8e4
I32 = mybir.dt.int32
OUT_BF16 = True
AF = mybir.ActivationFunctionType
DR = mybir.MatmulPerfMode.DoubleRow

# error-corrected fp8 head: logits = xh@wh + xl@wh + xh@wl, DoubleRow matmuls.
# h2 is produced pre-scaled by SX (folded into LN2's rsqrt); Wh is pre-scaled
# by SW on the host; the eviction copy divides by SX*SW.
SX = 8.0
SW = 32.0
INV_SXSW = 1.0 / (SX * SW)


def _window_start(i: int) -> int:
    # k-window [ws, ws+512) covers all allowed keys for query chunk i
    # (max lookback is WIN*GW + WIN = 297 < 384).
    return 128 * max(0, i - 3)


def _mask_tiles() -> np.ndarray:
    idx = np.arange(SEQ)
    r, c = idx // GW, idx % GW
    allow = (
        (np.abs(r[:, None] - r[None, :]) <= WIN)
        & (np.abs(c[:, None] - c[None, :]) <= WIN)
        & (idx[None, :] <= idx[:, None])
    )
    maskf = np.where(allow, 0.0, NEG).astype(np.float32)
    tiles = np.empty((NT, P, 512), np.float32)
    for i in range(NT):
        ws = _window_start(i)
        tiles[i] = maskf[i * P : (i + 1) * P, ws : ws + 512]
    return tiles


def _r(ap):
    """bitcast to float32r for full-rate fp32 matmul."""
    return ap.bitcast(F32R)


def _bcast_ap(a: bass.AP) -> bass.AP:
    """[n] DRAM vector AP -> [P, n] partition-broadcast DMA source."""
    return bass.AP(tensor=a.tensor, offset=a.offset, ap=[[0, P], *a.ap])


def _build_program(flags: dict, wh_bufs: int = 8, msk_bufs: int = 6, lean: bool = False) -> bass.Bass:
    nc = bacc.Bacc("TRN2", target_bir_lowering=False)

    # ---------- I/O ----------
    # x is supplied pre-transposed [P, NT] so its DMA is 32B-contiguous rows
    x_d = nc.declare_dram_parameter("x", [P, NT], I32, False)
    emb_d = nc.declare_dram_parameter("emb", [VOCAB, D], F32, False)
    pos_d = nc.declare_dram_parameter("pos", [SEQ, D], F32, False)
    msk_d = nc.declare_dram_parameter("maskt", [NT, P, 512], F32, False)
    wq_d = nc.declare_dram_parameter("wq", [D, D], F32, False)
    wk_d = nc.declare_dram_parameter("wk", [D, D], F32, False)
    wv_d = nc.declare_dram_parameter("wv", [D, D], F32, False)
    wo_d = nc.declare_dram_parameter("wo", [D, D], F32, False)
    w1_d = nc.declare_dram_parameter("w1", [D, DFF], F32, False)
    w2_d = nc.declare_dram_parameter("w2", [DFF, D], F32, False)
    whh_d = nc.declare_dram_parameter("whh", [D, VOCAB], F8, False)
    whl_d = nc.declare_dram_parameter("whl", [D, VOCAB], F8, False)
    dp = lambda name, shape: nc.declare_dram_parameter(name, shape, F32, False)
    bq_d = dp("bq", [D]) if flags["bq"] else None
    bk_d = dp("bk", [D]) if flags["bk"] else None
    bv_d = dp("bv", [D]) if flags["bv"] else None
    bo_d = dp("bo", [D]) if flags["bo"] else None
    b1_d = dp("b1", [DFF]) if flags["b1"] else None
    b2_d = dp("b2", [D]) if flags["b2"] else None
    bh_d = dp("bh", [VOCAB]) if flags["bh"] else None
    g1_d = dp("g1", [D]) if flags["g1"] else None
    be1_d = dp("be1", [D]) if flags["be1"] else None
    g2_d = dp("g2", [D]) if flags["g2"] else None
    be2_d = dp("be2", [D]) if flags["be2"] else None
    out_d = nc.declare_dram_parameter("out", [SEQ, VOCAB], BF16 if OUT_BF16 else F32, True)

    with tile.TileContext(nc) as tc:
        # ----- whole-kernel pools -----
        const = tc.alloc_tile_pool(name="const", bufs=1)
        small = tc.alloc_tile_pool(name="small", bufs=8)
        psum = tc.alloc_tile_pool(name="psA", bufs=6, space="PSUM")
        psum_t = tc.alloc_tile_pool(name="psT", bufs=2, space="PSUM")
        opool = tc.alloc_tile_pool(name="outev", bufs=2, side="right")
        p_h2T = tc.alloc_tile_pool(name="h2Tp", bufs=1, side="right")

        ident_f = const.tile([P, P], F32, tag="ident_f")
        ident = const.tile([P, P], F32R, tag="ident")
        eps_t = const.tile([P, 1], F32, tag="eps")
        nc.vector.memset(eps_t[:], EPS)
        # eps for LN2 with the SX scale folded in: sqrt((var+eps)/SX^2)
        eps2_t = const.tile([P, 1], F32, tag="eps2")
        nc.vector.memset(eps2_t[:], EPS / (SX * SX))
        x_sb = const.tile([P, NT], I32, tag="x_sb")
        nc.sync.dma_start(out=x_sb[:], in_=x_d[:])

        def load_col_bias(handle, nchunks, tag):
            # [nchunks*P] DRAM -> [P, nchunks] (chunk m in column m)
            t = const.tile([P, nchunks], F32, tag=tag)
            nc.sync.dma_start(out=t[:], in_=handle[:].rearrange("(m p) -> p m", p=P))
            return t

        def load_bcast(handle, n, tag):
            t = const.tile([P, n], F32, tag=tag)
            nc.sync.dma_start(out=t[:], in_=_bcast_ap(handle[:]))
            return t

        bq_sb = load_col_bias(bq_d, DC, "bq") if bq_d else None
        bk_sb = load_col_bias(bk_d, DC, "bk") if bk_d else None
        b1_sb = load_col_bias(b1_d, FC, "b1") if b1_d else None
        bv_bc = load_bcast(bv_d, D, "bv") if bv_d else None
        bo_bc = load_bcast(bo_d, D, "bo") if bo_d else None
        b2_bc = load_bcast(b2_d, D, "b2") if b2_d else None
        g1_bc = load_bcast(g1_d, D, "g1") if g1_d else None
        be1_bc = load_bcast(be1_d, D, "be1") if be1_d else None
        g2_bc = load_bcast(g2_d, D, "g2") if g2_d else None
        be2_bc = load_bcast(be2_d, D, "be2") if be2_d else None

        h2Th = [p_h2T.tile([P, DC, P], F8, tag=f"h2Th{j}", name=f"h2Th{j}") for j in range(NT)]
        h2Tl = [p_h2T.tile([P, DC, P], F8, tag=f"h2Tl{j}", name=f"h2Tl{j}") for j in range(NT)]

        # ----- phase A pools (left, LIFO) -----
        p_woh = tc.alloc_tile_pool(name="woh", bufs=1)         # wo, h  (-> stage 4)
        wo_sb = p_woh.tile([P, DC, D], F32R, tag="wo")
        h_sb = p_woh.tile([P, NT, D], F32R, tag="h")

        p_oT = tc.alloc_tile_pool(name="oTp", bufs=1)          # oT    (-> stage 4)
        oT = p_oT.tile([P, DC, SEQ], F32R, tag="oT")

        p_v = tc.alloc_tile_pool(name="vp", bufs=1)            # v (-> wave 2)
        v_sb = p_v.tile([P, NT, D], F32R, tag="v")
        p_at = tc.alloc_tile_pool(name="attnw", bufs=3)        # softmax work (-> stage 4)
        p_qk = tc.alloc_tile_pool(name="qkp", bufs=1)          # qT,kT (-> wave 1)
        qT = p_qk.tile([P, DC, SEQ], F32R, tag="qT")
        kT = p_qk.tile([P, DC, SEQ], F32R, tag="kT")

        p_wq = tc.alloc_tile_pool(name="wqp", bufs=1)          # wq,wk,wv,hT (-> stage 2)
        wq_sb = p_wq.tile([P, DC, D], F32R, tag="wq")
        wk_sb = p_wq.tile([P, DC, D], F32R, tag="wk")
        wv_sb = p_wq.tile([P, DC, D], F32R, tag="wv")
        hT = p_wq.tile([P, DC, SEQ], F32R, tag="hT")

        # ---------- stage 1: embedding gather + positional + transpose ----------
        for jj in range(NT):
            nc.gpsimd.indirect_dma_start(
                out=h_sb[:, jj, :],
                out_offset=None,
                in_=_r(emb_d[:]),
                in_offset=bass.IndirectOffsetOnAxis(ap=x_sb[:, jj : jj + 1], axis=0),
            )

        make_identity(nc, ident_f[:])
        nc.vector.tensor_copy(out=ident[:], in_=ident_f[:])

        def s1_add(j):
            pos_t = p_wq.tile([P, D], F32, tag="pos", bufs=3, name=f"pos{j}")
            nc.sync.dma_start(out=pos_t[:], in_=pos_d[j * P : (j + 1) * P, :])
            nc.vector.tensor_add(out=h_sb[:, j, :], in0=h_sb[:, j, :], in1=pos_t[:])

        def s1_trans(j):
            pt = psum_t.tile([P, 512], F32, tag="pt", name=f"s1pt{j}")
            for m in range(DC):
                nc.tensor.transpose(
                    out=_r(pt[:, m * P : (m + 1) * P]),
                    in_=_r(h_sb[:, j, m * P : (m + 1) * P]),
                    identity=_r(ident[:]),
                )
            nc.scalar.copy(out=hT[:, :, j * P : (j + 1) * P], in_=pt[:])

        for k in range(NT + 1):
            if k < NT:
                s1_add(k)
            if k >= 1:
                s1_trans(k - 1)

        # weight DMAs issued after stage-1 loads so embeddings/pos win the queue
        for w_sb, w_d in ((wq_sb, wq_d), (wk_sb, wk_d), (wv_sb, wv_d), (wo_sb, wo_d)):
            nc.sync.dma_start(out=w_sb[:], in_=_r(w_d[:].rearrange("(k p) o -> p k o", p=P)))

        # ---------- stage 2: qT / kT (d-major), v (token-major) ----------
        # t-major order: all groups needing hT[0:512] first (PE is in-order)
        for t in range(SEQ // 512):
            for (wt, bt, dst) in ((wq_sb, bq_sb, qT), (wk_sb, bk_sb, kT)):
                for m in range(DC):
                    ps = psum.tile([P, 512], F32, tag="ps")
                    for ki in range(DC):
                        nc.tensor.matmul(
                            ps[:],
                            _r(wt[:, ki, m * P : (m + 1) * P]),
                            _r(hT[:, ki, t * 512 : (t + 1) * 512]),
                            start=(ki == 0),
                            stop=(ki == DC - 1),
                        )
                    dslc = dst[:, m, t * 512 : (t + 1) * 512]
                    if bt is not None:
                        nc.scalar.activation(
                            out=dslc, in_=ps[:], func=AF.Identity,
                            bias=bt[:, m : m + 1], scale=1.0,
                        )
                    elif dst is kT:
                        nc.vector.tensor_copy(out=dslc, in_=ps[:])
                    else:
                        nc.scalar.copy(out=dslc, in_=ps[:])
            for j in range(4 * t, 4 * t + 4):
                ps = psum.tile([P, 512], F32, tag="ps")
                for ki in range(DC):
                    nc.tensor.matmul(
                        ps[:],
                        _r(hT[:, ki, j * P : (j + 1) * P]),
                        _r(wv_sb[:, ki, :]),
                        start=(ki == 0),
                        stop=(ki == DC - 1),
                    )
                if bv_bc is not None:
                    nc.vector.tensor_add(out=v_sb[:, j, :], in0=ps[:], in1=bv_bc[:])
                else:
                    nc.vector.tensor_copy(out=v_sb[:, j, :], in_=ps[:])

        p_wq.release()

        # ---------- stage 3 wave 1: scores + softmax ----------
        attns = []
        recips = []
        for i in range(NT):
            ws = _window_start(i)
            nw = min(512, max(256, (i + 1) * P))  # live window (>=256 keeps f32r fast)
            ps_s = psum.tile([P, 512], F32, tag="ps")
            for ki in range(DC):
                nc.tensor.matmul(
                    ps_s[:, :nw],
                    _r(qT[:, ki, i * P : (i + 1) * P]),
                    _r(kT[:, ki, ws : ws + nw]),
                    start=(ki == 0),
                    stop=(ki == DC - 1),
                )
            msk_t = p_at.tile([P, 512], F32, tag="msk", bufs=msk_bufs)
            nc.sync.dma_start(out=msk_t[:], in_=msk_d[i])
            s_t = p_at.tile([P, 512], F32, tag="s_t", bufs=3)
            nc.vector.tensor_add(out=s_t[:, :nw], in0=ps_s[:, :nw], in1=msk_t[:, :nw])
            attn = p_at.tile([P, 512], F32R, tag="attn", bufs=NT, name=f"attn{i}")
            denom = small.tile([P, 1], F32, tag="denom")
            nc.scalar.activation(
                out=attn[:, :nw], in_=s_t[:, :nw], func=AF.Exp,
                bias=0.0, scale=INV_SQRT_D,
                accum_out=denom[:, 0:1],
            )
            recip = small.tile([P, 1], F32, tag="recip", bufs=NT, name=f"recip{i}")
            nc.vector.reciprocal(out=recip[:], in_=denom[:])
            attns.append(attn)
            recips.append(recip)

        p_qk.release()

        # ----- right-side pools for FFN phase -----
        whpool = tc.alloc_tile_pool(name="whstream", bufs=wh_bufs, side="right")
        p_h1 = tc.alloc_tile_pool(name="h1p", bufs=1, side="right")
        h1_sb = p_h1.tile([P, NT, D], F32R, tag="h1")
        h1T = p_h1.tile([P, DC, SEQ], F32R, tag="h1T")
        w1_sb = p_h1.tile([P, DC, DFF], F32R, tag="w1")
        nc.sync.dma_start(out=w1_sb[:], in_=_r(w1_d[:].rearrange("(k p) o -> p k o", p=P)))

        # ---------- stage 3 wave 2 + stage 4, software-pipelined ----------
        p_st4 = tc.alloc_tile_pool(name="st4", bufs=3)
        attnTs = [None] * NT
        o_ts = [None] * NT

        def w2_a(i):  # attn transposes + attnT eviction
            ws = _window_start(i)
            kb0 = ws // P
            nkb = min(DC, i - kb0 + 1)
            pt = psum_t.tile([P, 512], F32, tag="pt", name=f"atp{i}")
            for kk in range(nkb):
                nc.tensor.transpose(
                    out=_r(pt[:, kk * P : (kk + 1) * P]),
                    in_=_r(attns[i][:, kk * P : (kk + 1) * P]),
                    identity=_r(ident[:]),
                )
            attnT = p_at.tile([P, 512], F32R, tag="attnT", bufs=3, name=f"attnT{i}")
            nc.scalar.copy(out=attnT[:, : nkb * P], in_=pt[:, : nkb * P])
            attnTs[i] = attnT

        def w2_b(i):  # o matmuls + scale
            ws = _window_start(i)
            kb0 = ws // P
            nkb = min(DC, i - kb0 + 1)
            ps_o = psum.tile([P, 512], F32, tag="ps", name=f"pso{i}")
            for kk in range(nkb):
                nc.tensor.matmul(
                    ps_o[:],
                    attnTs[i][:, kk * P : (kk + 1) * P],
                    _r(v_sb[:, kb0 + kk, :]),
                    start=(kk == 0),
                    stop=(kk == nkb - 1),
                )
            o_t = p_at.tile([P, D], F32R, tag="o_t", bufs=3, name=f"o_t{i}")
            nc.vector.tensor_scalar_mul(out=o_t[:], in0=ps_o[:], scalar1=recips[i][:, 0:1])
            o_ts[i] = o_t

        def w2_c(i):  # oT transposes + eviction
            pt2 = psum_t.tile([P, 512], F32, tag="pt", name=f"otp{i}")
            for m in range(DC):
                nc.tensor.transpose(
                    out=_r(pt2[:, m * P : (m + 1) * P]),
                    in_=_r(o_ts[i][:, m * P : (m + 1) * P]),
                    identity=_r(ident[:]),
                )
            nc.vector.tensor_copy(out=oT[:, :, i * P : (i + 1) * P], in_=pt2[:])

        def s4_proj(j):  # attn projection + residual + LN1 (no transpose)
            ps = psum.tile([P, 512], F32, tag="ps", name=f"psp{j}")
            for m in range(DC):
                nc.tensor.matmul(
                    ps[:],
                    _r(oT[:, m, j * P : (j + 1) * P]),
                    _r(wo_sb[:, m, :]),
                    start=(m == 0),
                    stop=(m == DC - 1),
                )
            r1 = p_st4.tile([P, D], F32, tag="r1", name=f"r1_{j}")
            nc.vector.tensor_add(out=r1[:], in0=h_sb[:, j, :], in1=ps[:])
            if bo_bc is not None:
                nc.vector.tensor_add(out=r1[:], in0=r1[:], in1=bo_bc[:])
            stats = small.tile([P, 6], F32, tag="stats")
            nc.vector.bn_stats(out=stats[:], in_=r1[:])
            mv = small.tile([P, 2], F32, tag="mv")
            nc.vector.bn_aggr(out=mv[:], in_=stats[:])
            stdt = small.tile([P, 1], F32, tag="stdt")
            nc.scalar.activation(
                out=stdt[:], in_=mv[:, 1:2], func=AF.Sqrt,
                bias=eps_t[:, 0:1], scale=1.0,
            )
            rstd = small.tile([P, 1], F32, tag="rstd")
            nc.vector.reciprocal(out=rstd[:], in_=stdt[:])
            nc.vector.tensor_scalar(
                out=h1_sb[:, j, :], in0=r1[:],
                scalar1=mv[:, 0:1], scalar2=rstd[:, 0:1],
                op0=mybir.AluOpType.subtract, op1=mybir.AluOpType.mult,
            )
            if g1_bc is not None:
                nc.vector.tensor_mul(out=h1_sb[:, j, :], in0=h1_sb[:, j, :], in1=g1_bc[:])
            if be1_bc is not None:
                nc.vector.tensor_add(out=h1_sb[:, j, :], in0=h1_sb[:, j, :], in1=be1_bc[:])

        def s4_trans(j):  # h1 transposes + h1T eviction
            pt3 = psum_t.tile([P, 512], F32, tag="pt", name=f"h1p{j}")
            for m in range(DC):
                nc.tensor.transpose(
                    out=_r(pt3[:, m * P : (m + 1) * P]),
                    in_=_r(h1_sb[:, j, m * P : (m + 1) * P]),
                    identity=_r(ident[:]),
                )
            nc.scalar.copy(out=h1T[:, :, j * P : (j + 1) * P], in_=pt3[:])

        for k in range(NT + 4):
            if k < NT:
                w2_a(k)
            if 1 <= k < NT + 1:
                w2_b(k - 1)
            if 2 <= k < NT + 2:
                w2_c(k - 2)
            if 3 <= k < NT + 3:
                s4_proj(k - 3)
            if 4 <= k:
                s4_trans(k - 4)

        p_st4.release()
        p_at.release()
        p_v.release()
        p_oT.release()
        p_woh.release()

        p_w12 = tc.alloc_tile_pool(name="w12", bufs=1, side="right")
        w2_sb = p_w12.tile([P, FC, D], F32R, tag="w2")
        nc.sync.dma_start(out=w2_sb[:], in_=_r(w2_d[:].rearrange("(k p) o -> p k o", p=P)))

        # ---------- stage 5: FFN up, f1T = relu(W1^T @ h1T + b1) ----------
        p_f1 = tc.alloc_tile_pool(name="f1p", bufs=1, side="right")
        f1T = p_f1.tile([P, FC, SEQ], F32R, tag="f1T")
        def ffn1_group(n, t):
            ps = psum.tile([P, 512], F32, tag="ps", name=f"psf{n}_{t}")
            for ki in range(DC):
                nc.tensor.matmul(
                    ps[:],
                    _r(w1_sb[:, ki, n * P : (n + 1) * P]),
                    _r(h1T[:, ki, t * 512 : (t + 1) * 512]),
                    start=(ki == 0),
                    stop=(ki == DC - 1),
                )
            fslc = f1T[:, n, t * 512 : (t + 1) * 512]
            if b1_sb is not None:
                nc.vector.tensor_scalar(
                    out=fslc, in0=ps[:],
                    scalar1=b1_sb[:, n : n + 1], scalar2=0.0,
                    op0=mybir.AluOpType.add, op1=mybir.AluOpType.max,
                )
            else:
                nc.vector.tensor_scalar_max(out=fslc, in0=ps[:], scalar1=0.0)

        # ---------- stage 6: FFN down + residual + LN2 (pipelined) ----------
        def s6_main(j):
            ps = psum.tile([P, 512], F32, tag="ps", name=f"ps6_{j}")
            for n in range(FC):
                nc.tensor.matmul(
                    ps[:],
                    _r(f1T[:, n, j * P : (j + 1) * P]),
                    _r(w2_sb[:, n, :]),
                    start=(n == 0),
                    stop=(n == FC - 1),
                )
            r2 = p_f1.tile([P, D], F32, tag="r2", bufs=3, name=f"r2_{j}")
            nc.vector.tensor_add(out=r2[:], in0=h1_sb[:, j, :], in1=ps[:])
            if b2_bc is not None:
                nc.vector.tensor_add(out=r2[:], in0=r2[:], in1=b2_bc[:])
            stats = small.tile([P, 6], F32, tag="stats")
            nc.vector.bn_stats(out=stats[:], in_=r2[:])
            mv = small.tile([P, 2], F32, tag="mv")
            nc.vector.bn_aggr(out=mv[:], in_=stats[:])
            stdt = small.tile([P, 1], F32, tag="stdt")
            # stdt = sqrt((var+eps))/SX so the LN output comes out x SX
            nc.scalar.activation(
                out=stdt[:], in_=mv[:, 1:2], func=AF.Sqrt,
                bias=eps2_t[:, 0:1], scale=1.0 / (SX * SX),
            )
            rstd = small.tile([P, 1], F32, tag="rstd")
            nc.vector.reciprocal(out=rstd[:], in_=stdt[:])
            h2_t = p_f1.tile([P, D], F32R, tag="h2_t", bufs=3, name=f"h2t_{j}")
            nc.vector.tensor_scalar(
                out=h2_t[:], in0=r2[:],
                scalar1=mv[:, 0:1], scalar2=rstd[:, 0:1],
                op0=mybir.AluOpType.subtract, op1=mybir.AluOpType.mult,
            )
            if g2_bc is not None:
                nc.vector.tensor_mul(out=h2_t[:], in0=h2_t[:], in1=g2_bc[:])
            if be2_bc is not None:
                nc.vector.tensor_add(out=h2_t[:], in0=h2_t[:], in1=be2_bc[:])
            return h2_t

        h2ts = [None] * NT

        def s6_trans(j):
            pt = psum_t.tile([P, 512], F32, tag="pt", name=f"h2p{j}")
            for m in range(DC):
                nc.tensor.transpose(
                    out=_r(pt[:, m * P : (m + 1) * P]),
                    in_=_r(h2ts[j][:, m * P : (m + 1) * P]),
                    identity=_r(ident[:]),
                )
            # fp8 split: hi = q8(h2T), lo = q8(h2T - hi)
            nc.scalar.copy(out=h2Th[j][:, :, :], in_=pt[:])
            nc.vector.tensor_sub(
                out=h2Tl[j][:, :, :], in0=pt[:], in1=h2Th[j][:, :, :]
            )

        # head chunks for vc=0,1 interleaved into stage-6 so PE fills LN waits
        whh_r = whh_d[:].rearrange("(k p) v -> p k v", p=P)
        whl_r = whl_d[:].rearrange("(k p) v -> p k v", p=P)

        def load_whv(vc, name):
            wh = whpool.tile([P, DC, 512], F8, tag="whv", name=f"{name}h")
            nc.sync.dma_start(out=wh[:], in_=whh_r[:, :, vc * 512 : (vc + 1) * 512])
            wl = whpool.tile([P, DC, 512], F8, tag="whv", name=f"{name}l")
            nc.sync.dma_start(out=wl[:], in_=whl_r[:, :, vc * 512 : (vc + 1) * 512])
            return wh, wl

        whv0 = load_whv(0, "whv0")
        otile0 = opool.tile([P, NT, 512], BF16 if OUT_BF16 else F32, tag="ot", name="otile0")
        whv1 = load_whv(1, "whv1")
        otile1 = opool.tile([P, NT, 512], BF16 if OUT_BF16 else F32, tag="ot", name="otile1")

        def head_j(whv, otile, j, toggle):
            wh, wl = whv
            ps = psum.tile([P, 512], F32, tag="ps", name=f"psh{toggle}_{j}")
            # 3-term error-corrected fp8, all DoubleRow (contract 256/instr):
            #   xh@wh + xl@wh + xh@wl
            terms = ((h2Th[j], wh), (h2Tl[j], wh), (h2Th[j], wl))
            nterm = len(terms)
            for t_i, (xt, wt) in enumerate(terms):
                for k2 in range(DC // 2):
                    nc.tensor.matmul(
                        ps[:],
                        xt[:, 2 * k2 : 2 * k2 + 2, :],
                        wt[:, 2 * k2 : 2 * k2 + 2, :],
                        start=(t_i == 0 and k2 == 0),
                        stop=(t_i == nterm - 1 and k2 == DC // 2 - 1),
                        perf_mode=DR,
                    )
            if bh_sb_for(toggle) is not None:
                sc = whpool.tile([P, 512], F32, tag="hsc", bufs=2, name=f"hsc{toggle}_{j}")
                nc.scalar.activation(
                    out=sc[:], in_=ps[:], func=AF.Identity, bias=0.0, scale=INV_SXSW,
                )
                nc.vector.tensor_add(out=otile[:, j, :], in0=sc[:], in1=bh_sb_for(toggle)[:])
            elif j % 2 == 0:
                nc.vector.tensor_scalar_mul(out=otile[:, j, :], in0=ps[:], scalar1=INV_SXSW)
            else:
                nc.scalar.activation(
                    out=otile[:, j, :], in_=ps[:], func=AF.Identity,
                    bias=0.0, scale=INV_SXSW,
                )

        _bh_tiles = {}

        def bh_sb_for(key):
            return _bh_tiles.get(key)

        if bh_d is not None:
            bh0 = whpool.tile([P, 512], F32, tag="bh", bufs=2, name="bh0")
            nc.sync.dma_start(out=bh0[:], in_=_bcast_ap(bh_d[0:512]))
            _bh_tiles[0] = bh0
            bh1 = whpool.tile([P, 512], F32, tag="bh", bufs=2, name="bh1")
            nc.sync.dma_start(out=bh1[:], in_=_bcast_ap(bh_d[512:1024]))
            _bh_tiles[1] = bh1

        for t in range(SEQ // 512):
            for n in range(FC):
                ffn1_group(n, t)
                if t == 1 and n % 2 == 1:
                    j = n // 2
                    h2ts[j] = s6_main(j)

        for k in range(NT + 3):
            if 4 <= k < NT:
                h2ts[k] = s6_main(k)
            if 1 <= k <= NT:
                s6_trans(k - 1)
            if 2 <= k <= NT + 1:
                head_j(whv0, otile0, k - 2, 0)
            if 3 <= k <= NT + 2:
                head_j(whv1, otile1, k - 3, 1)
        out_rr = out_d[:].rearrange("(j p) v -> p j v", p=P)
        nc.sync.dma_start(out=out_rr[:, :, 0:512], in_=otile0[:])
        nc.sync.dma_start(out=out_rr[:, :, 512:1024], in_=otile1[:])

        p_f1.release()
        p_w12.release()
        p_h1.release()

        # ---------- stage 7: vocab head (vc >= 2) ----------
        out_r = out_d[:].rearrange("(j p) v -> p j v", p=P)
        for vc in range(2, NV):
            whv = load_whv(vc, f"whv{vc}")
            if bh_d is not None:
                bh_bc = whpool.tile([P, 512], F32, tag="bh", bufs=2, name=f"bh{vc}")
                nc.sync.dma_start(
                    out=bh_bc[:], in_=_bcast_ap(bh_d[vc * 512 : (vc + 1) * 512])
                )
                _bh_tiles[vc] = bh_bc
            otile = opool.tile([P, NT, 512], BF16 if OUT_BF16 else F32, tag="ot")
            for j in range(NT):
                head_j(whv, otile, j, vc)
                # split the store so the tail drain overlaps the evictions
                if j == NT // 2 - 1:
                    nc.sync.dma_start(
                        out=out_r[:, 0 : NT // 2, vc * 512 : (vc + 1) * 512],
                        in_=otile[:, 0 : NT // 2, :],
                    )
            nc.sync.dma_start(
                out=out_r[:, NT // 2 :, vc * 512 : (vc + 1) * 512],
                in_=otile[:, NT // 2 :, :],
            )

        whpool.release()
        p_h2T.release()
        opool.release()
        psum_t.release()
        psum.release()
        small.release()
        const.release()

    nc.finalize()
    return nc


_PROGRAM_CACHE: dict = {}


def _get_program(flags: dict) -> bass.Bass:
    key = tuple(sorted(flags.items()))
    if key not in _PROGRAM_CACHE:
        _PROGRAM_CACHE[key] = _build_program(flags)
    return _PROGRAM_CACHE[key]


def _prep(x, embed_tab, row_embed, col_embed, Wq, bq, Wk, bk, Wv, bv, Wo, bo,
          ln1_g, ln1_b, W1, b1, W2, b2, ln2_g, ln2_b, Wh, bh):
    """Shared host-side prep: flags, common input map, per-core x shards."""
    f32c = lambda a: np.ascontiguousarray(np.asarray(a, dtype=np.float32))
    x = np.asarray(x)
    B = x.shape[0]
    assert x.shape == (B, SEQ)

    arrs = dict(
        emb=f32c(embed_tab), wq=f32c(Wq), wk=f32c(Wk), wv=f32c(Wv), wo=f32c(Wo),
        w1=f32c(W1), w2=f32c(W2),
    )
    import ml_dtypes
    whs = f32c(Wh) * np.float32(SW)
    whh = whs.astype(ml_dtypes.float8_e4m3)
    whl = (whs - whh.astype(np.float32)).astype(ml_dtypes.float8_e4m3)
    arrs["whh"] = np.ascontiguousarray(whh)
    arrs["whl"] = np.ascontiguousarray(whl)
    pos = np.concatenate(
        [np.repeat(f32c(row_embed), GW, axis=0), np.tile(f32c(col_embed), (GH, 1))],
        axis=-1,
    )
    arrs["pos"] = np.ascontiguousarray(pos, dtype=np.float32)
    arrs["maskt"] = _mask_tiles()

    bias_map = dict(
        bq=f32c(bq), bk=f32c(bk), bv=f32c(bv), bo=f32c(bo), b1=f32c(b1),
        b2=f32c(b2), bh=f32c(bh),
        be1=f32c(ln1_b),
        # LN2's output is produced pre-scaled by SX; its bias must match
        be2=f32c(ln2_b) * np.float32(SX),
    )
    gain_map = dict(g1=f32c(ln1_g), g2=f32c(ln2_g))
    flags = {k: bool(np.any(v)) for k, v in bias_map.items()}
    flags.update({k: bool(np.any(v != 1.0)) for k, v in gain_map.items()})
    for k, v in {**bias_map, **gain_map}.items():
        if flags[k]:
            arrs[k] = v

    # pre-transpose x to [P, NT] so the device DMA has 32B-contiguous rows
    xs = [
        np.ascontiguousarray(x[c].astype(np.int32).reshape(NT, P).T)
        for c in range(B)
    ]
    return flags, arrs, xs, B


def kernel(**inputs):
    flags, arrs, xs, B = _prep(**inputs)
    nc = _get_program(flags)
    core_ids = list(range(8))
    in_maps = [{**arrs, "x": xs[c % B]} for c in core_ids]
    res = run_bass_kernel_spmd(nc, in_maps, core_ids)
    out = np.stack([res.results[c]["out"] for c in range(B)], axis=0)
    return np.asarray(out, dtype=np.float32)



# revision 54
# speedup vs baseline: 1.3445x; 1.1382x over previous
"""Trainium2 Bass kernel for a small autoregressive transformer block with
local-windowed causal attention and a large (16k) vocab head.

Data-parallel over batch: batch item b runs on NeuronCore b (8 cores).
Per core:
  h   = embed_tab[x] + pos                      [1024, 512]
  q/k/v = h @ Wq/k/v (+b)                       [1024, 512]
  s   = q @ k^T / sqrt(D) + local_causal_mask   (banded, window <= 298)
  o   = softmax(s) @ v @ Wo (+bo)
  h1  = LN(h + o);  f = relu(h1@W1+b1)@W2+b2;  h2 = LN(h1 + f)
  out = h2 @ Wh (+bh)                           [1024, 16384]

All matmuls run as float32r (full-rate fp32 with N=512 moving dim).
kernel(**inputs) takes full unsharded inputs, returns [8, 1024, 16384] f32.
"""

import math
import numpy as np

import concourse.bass as bass
import concourse.mybir as mybir
import concourse.tile as tile
from concourse import bacc
from concourse.bass_utils import run_bass_kernel_spmd
from concourse.masks import make_identity

# ---- problem constants (hardcoded per contract) ----
GH = 32
GW = 32
SEQ = 1024
WIN = 9
D = 512
DFF = 1024
VOCAB = 16384
EPS = 1e-5
NEG = -1e30

P = 128
NT = SEQ // P        # 8 token chunks
DC = D // P          # 4 d chunks
FC = DFF // P        # 8 dff chunks
NV = VOCAB // 512    # 32 vocab chunks
INV_SQRT_D = 1.0 / math.sqrt(D)

F32 = mybir.dt.float32
F32R = mybir.dt.float32r
BF16 = mybir.dt.bfloat16
F8 = mybir.dt.float8e4
I32 = mybir.dt.int32
OUT_BF16 = True
AF = mybir.ActivationFunctionType
DR = mybir.MatmulPerfMode.DoubleRow

# error-corrected fp8 head: logits = xh@wh + xl@wh + xh@wl, DoubleRow matmuls.
# h2 is produced pre-scaled by SX (folded into LN2's rsqrt); Wh is pre-scaled
# by SW on the host; the eviction copy divides by SX*SW.
SX = 8.0
SW = 32.0
INV_SXSW = 1.0 / (SX * SW)


def _window_start(i: int) -> int:
    # k-window [ws, ws+512) covers all allowed keys for query chunk i
    # (max lookback is WIN*GW + WIN = 297 < 384).
    return 128 * max(0, i - 3)


def _mask_tiles() -> np.ndarray:
    idx = np.arange(SEQ)
    r, c = idx // GW, idx % GW
    allow = (
        (np.abs(r[:, None] - r[None, :]) <= WIN)
        & (np.abs(c[:, None] - c[None, :]) <= WIN)
        & (idx[None, :] <= idx[:, None])
    )
    maskf = np.where(allow, 0.0, NEG).astype(np.float32)
    tiles = np.empty((NT, P, 512), np.float32)
    for i in range(NT):
        ws = _window_start(i)
        tiles[i] = maskf[i * P : (i + 1) * P, ws : ws + 512]
    return tiles


def _r(ap):
    """bitcast to float32r for full-rate fp32 matmul."""
    return ap.bitcast(F32R)


def _bcast_ap(a: bass.AP) -> bass.AP:
    """[n] DRAM vector AP -> [P, n] partition-broadcast DMA source."""
    return bass.AP(tensor=a.tensor, offset=a.offset, ap=[[0, P], *a.ap])


def _build_program(flags: dict, wh_bufs: int = 16, msk_bufs: int = 6, lean: bool = False) -> bass.Bass:
    nc = bacc.Bacc("TRN2", target_bir_lowering=False)

    # ---------- I/O ----------
    # h = emb[x] + pos is gathered host-side and shipped in both layouts:
    # token-major (residual adds, f32) and d-major/transposed (matmul
    # operand, bf16 — quantization is well under the error budget)
    h_d = nc.declare_dram_parameter("h", [P, NT, D], F32, False)
    ht_d = nc.declare_dram_parameter("ht", [P, DC, SEQ], BF16, False)
    msk_d = nc.declare_dram_parameter("maskt", [NT, P, 512], BF16, False)
    # fused attention weights: A = Wq @ Wk^T / sqrt(D)  (scores = hA h^T),
    # B = Wv @ Wo  (o = attn @ (h B)); both computed host-side, shipped bf16.
    wa_d = nc.declare_dram_parameter("wa", [D, D], BF16, False)
    wb_d = nc.declare_dram_parameter("wb", [D, D], BF16, False)
    w1_d = nc.declare_dram_parameter("w1", [D, DFF], BF16, False)
    w2_d = nc.declare_dram_parameter("w2", [DFF, D], BF16, False)
    whh_d = nc.declare_dram_parameter("whh", [D, VOCAB], F8, False)
    whl_d = nc.declare_dram_parameter("whl", [D, VOCAB], F8, False)
    dp = lambda name, shape: nc.declare_dram_parameter(name, shape, F32, False)
    assert not (flags["bq"] or flags["bk"]), (
        "QK-fused path requires zero q/k biases (true for this problem)"
    )
    # bc = bv @ Wo + bo, folded into the attention-output residual add
    bc_d = dp("bc", [D]) if flags["bc"] else None
    b1_d = dp("b1", [DFF]) if flags["b1"] else None
    b2_d = dp("b2", [D]) if flags["b2"] else None
    bh_d = dp("bh", [VOCAB]) if flags["bh"] else None
    g1_d = dp("g1", [D]) if flags["g1"] else None
    be1_d = dp("be1", [D]) if flags["be1"] else None
    g2_d = dp("g2", [D]) if flags["g2"] else None
    be2_d = dp("be2", [D]) if flags["be2"] else None
    out_d = nc.declare_dram_parameter("out", [SEQ, VOCAB], BF16 if OUT_BF16 else F32, True)

    with tile.TileContext(nc) as tc:
        # ----- whole-kernel pools -----
        const = tc.alloc_tile_pool(name="const", bufs=1)
        small = tc.alloc_tile_pool(name="small", bufs=8)
        psum = tc.alloc_tile_pool(name="psA", bufs=5, space="PSUM")
        psum_t = tc.alloc_tile_pool(name="psT", bufs=2, space="PSUM")
        opool = tc.alloc_tile_pool(name="outev", bufs=2, side="right")
        p_h2T = tc.alloc_tile_pool(name="h2Tp", bufs=1, side="right")

        ident_f = const.tile([P, P], F32, tag="ident_f")
        ident = const.tile([P, P], F32R, tag="ident")
        eps_t = const.tile([P, 1], F32, tag="eps")
        nc.vector.memset(eps_t[:], EPS)
        # eps for LN2 with the SX scale folded in: sqrt((var+eps)/SX^2)
        eps2_t = const.tile([P, 1], F32, tag="eps2")
        nc.vector.memset(eps2_t[:], EPS / (SX * SX))

        def load_col_bias(handle, nchunks, tag):
            # [nchunks*P] DRAM -> [P, nchunks] (chunk m in column m)
            t = const.tile([P, nchunks], F32, tag=tag)
            nc.sync.dma_start(out=t[:], in_=handle[:].rearrange("(m p) -> p m", p=P))
            return t

        def load_bcast(handle, n, tag):
            t = const.tile([P, n], F32, tag=tag)
            nc.sync.dma_start(out=t[:], in_=_bcast_ap(handle[:]))
            return t

        b1_sb = load_col_bias(b1_d, FC, "b1") if b1_d else None
        bc_bc = load_bcast(bc_d, D, "bc") if bc_d else None
        b2_bc = load_bcast(b2_d, D, "b2") if b2_d else None
        g1_bc = load_bcast(g1_d, D, "g1") if g1_d else None
        be1_bc = load_bcast(be1_d, D, "be1") if be1_d else None
        g2_bc = load_bcast(g2_d, D, "g2") if g2_d else None
        be2_bc = load_bcast(be2_d, D, "be2") if be2_d else None

        h2Th = [p_h2T.tile([P, DC, P], F8, tag=f"h2Th{j}", name=f"h2Th{j}") for j in range(NT)]
        h2Tl = [p_h2T.tile([P, DC, P], F8, tag=f"h2Tl{j}", name=f"h2Tl{j}") for j in range(NT)]

        # ----- phase A pools (left, LIFO) -----
        p_woh = tc.alloc_tile_pool(name="woh", bufs=1)         # h  (-> stage 4)
        h_sb = p_woh.tile([P, NT, D], F32, tag="h")

        p_v = tc.alloc_tile_pool(name="vp", bufs=1)            # v (-> wave 2)
        v_sb = p_v.tile([P, NT, D], BF16, tag="v")
        p_at = tc.alloc_tile_pool(name="attnw", bufs=3)        # softmax work (-> stage 4)
        p_qk = tc.alloc_tile_pool(name="qkp", bufs=1)          # qAT,hT (-> wave 1)
        qT = p_qk.tile([P, DC, SEQ], BF16, tag="qT")
        hT = p_qk.tile([P, DC, SEQ], BF16, tag="hT")

        p_wq = tc.alloc_tile_pool(name="wqp", bufs=1)          # wa,wb (-> stage 2)
        wa_sb = p_wq.tile([P, DC, D], BF16, tag="wa")
        wb_sb = p_wq.tile([P, DC, D], BF16, tag="wb")

        # ---------- stage 1: load h (host-gathered emb[x]+pos) in both layouts
        # DMA order = stage-2 dependency order: hT half 0, wa, hT half 1, wb
        wa_r = wa_d[:].rearrange("(k p) o -> p k o", p=P)
        nc.sync.dma_start(out=wa_sb[:, :, 0:P], in_=wa_r[:, :, 0:P])
        nc.sync.dma_start(out=hT[:, :, 0:512], in_=ht_d[:, :, 0:512])
        nc.sync.dma_start(out=wa_sb[:, :, P:], in_=wa_r[:, :, P:])
        nc.sync.dma_start(out=hT[:, :, 512:1024], in_=ht_d[:, :, 512:1024])
        nc.sync.dma_start(out=wb_sb[:], in_=wb_d[:].rearrange("(k p) o -> p k o", p=P))
        nc.sync.dma_start(out=h_sb[:], in_=h_d[:])

        make_identity(nc, ident_f[:])
        nc.vector.tensor_copy(out=ident[:], in_=ident_f[:])
        ident_bf = const.tile([P, P], BF16, tag="ident_bf")
        nc.vector.tensor_copy(out=ident_bf[:], in_=ident_f[:])

        # ---------- stage 2: qAT = (hA)^T (d-major), v = hB (token-major) ----------
        # t-major order: all groups needing hT[0:512] first (PE is in-order)
        for t in range(SEQ // 512):
            for m in range(DC):
                ps = psum.tile([P, 512], F32, tag="ps")
                for ki in range(DC):
                    nc.tensor.matmul(
                        ps[:],
                        wa_sb[:, ki, m * P : (m + 1) * P],
                        hT[:, ki, t * 512 : (t + 1) * 512],
                        start=(ki == 0),
                        stop=(ki == DC - 1),
                    )
                dslc = qT[:, m, t * 512 : (t + 1) * 512]
                if m % 2 == 0:
                    nc.vector.tensor_copy(out=dslc, in_=ps[:])
                else:
                    nc.scalar.copy(out=dslc, in_=ps[:])
            for j in range(4 * t, 4 * t + 4):
                ps = psum.tile([P, 512], F32, tag="ps")
                for ki in range(DC):
                    nc.tensor.matmul(
                        ps[:],
                        hT[:, ki, j * P : (j + 1) * P],
                        wb_sb[:, ki, :],
                        start=(ki == 0),
                        stop=(ki == DC - 1),
                    )
                if j % 2 == 0:
                    nc.scalar.copy(out=v_sb[:, j, :], in_=ps[:])
                else:
                    nc.vector.tensor_copy(out=v_sb[:, j, :], in_=ps[:])

        p_wq.release()

        # ---------- stage 3 wave 1: scores + softmax ----------
        attns = []
        recips = []
        for i in range(NT):
            ws = _window_start(i)
            nw = min(512, max(256, (i + 1) * P))  # live window
            msk_t = p_at.tile([P, 512], BF16, tag="msk", bufs=msk_bufs)
            nc.sync.dma_start(out=msk_t[:], in_=msk_d[i])
            ps_s = psum.tile([P, 512], F32, tag="ps")
            for ki in range(DC):
                nc.tensor.matmul(
                    ps_s[:, :nw],
                    qT[:, ki, i * P : (i + 1) * P],
                    hT[:, ki, ws : ws + nw],
                    start=(ki == 0),
                    stop=False,
                )
            # additive mask folded into the PSUM group: ps += I^T @ mask
            nc.tensor.matmul(
                ps_s[:, :nw], ident_bf[:], msk_t[:, :nw], start=False, stop=True,
            )
            attn = p_at.tile([P, 512], BF16, tag="attn", bufs=NT, name=f"attn{i}")
            denom = small.tile([P, 1], F32, tag="denom")
            # A carries the 1/sqrt(D) factor, so the scores arrive pre-scaled
            nc.scalar.activation(
                out=attn[:, :nw], in_=ps_s[:, :nw], func=AF.Exp,
                bias=0.0, scale=1.0,
                accum_out=denom[:, 0:1],
            )
            recip = small.tile([P, 1], F32, tag="recip", bufs=NT, name=f"recip{i}")
            nc.vector.reciprocal(out=recip[:], in_=denom[:])
            attns.append(attn)
            recips.append(recip)

        p_qk.release()

        # ----- right-side pools for FFN phase -----
        whpool = tc.alloc_tile_pool(name="whstream", bufs=wh_bufs, side="right")
        p_h1 = tc.alloc_tile_pool(name="h1p", bufs=1, side="right")
        h1_sb = p_h1.tile([P, NT, D], F32R, tag="h1")
        h1T = p_h1.tile([P, DC, SEQ], BF16, tag="h1T")
        w1_sb = p_h1.tile([P, DC, DFF], BF16, tag="w1")
        nc.sync.dma_start(out=w1_sb[:], in_=w1_d[:].rearrange("(k p) o -> p k o", p=P))

        # ---------- stage 3 wave 2 + stage 4, software-pipelined ----------
        p_st4 = tc.alloc_tile_pool(name="st4", bufs=3)
        attnTs = [None] * NT
        o_ts = [None] * NT

        def w2_a(i):  # attn transposes (bf16) + attnT eviction
            ws = _window_start(i)
            kb0 = ws // P
            nkb = min(DC, i - kb0 + 1)
            pt = psum_t.tile([P, 512], BF16, tag="ptb", bufs=2, name=f"atp{i}")
            for kk in range(nkb):
                nc.tensor.transpose(
                    out=pt[:, kk * P : (kk + 1) * P],
                    in_=attns[i][:, kk * P : (kk + 1) * P],
                    identity=ident_bf[:],
                )
            attnT = p_at.tile([P, 512], BF16, tag="attnT", bufs=3, name=f"attnT{i}")
            nc.scalar.copy(out=attnT[:, : nkb * P], in_=pt[:, : nkb * P])
            attnTs[i] = attnT

        def w2_b(i):  # o matmuls + scale
            ws = _window_start(i)
            kb0 = ws // P
            nkb = min(DC, i - kb0 + 1)
            ps_o = psum.tile([P, 512], F32, tag="ps", name=f"pso{i}")
            for kk in range(nkb):
                nc.tensor.matmul(
                    ps_o[:],
                    attnTs[i][:, kk * P : (kk + 1) * P],
                    v_sb[:, kb0 + kk, :],
                    start=(kk == 0),
                    stop=(kk == nkb - 1),
                )
            o_t = p_at.tile([P, D], F32, tag="o_t", bufs=3, name=f"o_t{i}")
            nc.vector.tensor_scalar_mul(out=o_t[:], in0=ps_o[:], scalar1=recips[i][:, 0:1])
            o_ts[i] = o_t

        def s4_ln(j):  # residual + LN1 (o is already fully projected via B)
            r1 = p_st4.tile([P, D], F32, tag="r1", name=f"r1_{j}")
            nc.vector.tensor_add(out=r1[:], in0=h_sb[:, j, :], in1=o_ts[j][:])
            if bc_bc is not None:
                nc.vector.tensor_add(out=r1[:], in0=r1[:], in1=bc_bc[:])
            stats = small.tile([P, 6], F32, tag="stats")
            nc.vector.bn_stats(out=stats[:], in_=r1[:])
            mv = small.tile([P, 2], F32, tag="mv")
            nc.vector.bn_aggr(out=mv[:], in_=stats[:])
            stdt = small.tile([P, 1], F32, tag="stdt")
            nc.scalar.activation(
                out=stdt[:], in_=mv[:, 1:2], func=AF.Sqrt,
                bias=eps_t[:, 0:1], scale=1.0,
            )
            rstd = small.tile([P, 1], F32, tag="rstd")
            nc.vector.reciprocal(out=rstd[:], in_=stdt[:])
            nc.vector.tensor_scalar(
                out=h1_sb[:, j, :], in0=r1[:],
                scalar1=mv[:, 0:1], scalar2=rstd[:, 0:1],
                op0=mybir.AluOpType.subtract, op1=mybir.AluOpType.mult,
            )
            if g1_bc is not None:
                nc.vector.tensor_mul(out=h1_sb[:, j, :], in0=h1_sb[:, j, :], in1=g1_bc[:])
            if be1_bc is not None:
                nc.vector.tensor_add(out=h1_sb[:, j, :], in0=h1_sb[:, j, :], in1=be1_bc[:])

        def s4_trans(j):  # h1 transposes + h1T eviction
            pt3 = psum_t.tile([P, 512], F32, tag="pt", bufs=1, name=f"h1p{j}")
            for m in range(DC):
                nc.tensor.transpose(
                    out=_r(pt3[:, m * P : (m + 1) * P]),
                    in_=_r(h1_sb[:, j, m * P : (m + 1) * P]),
                    identity=_r(ident[:]),
                )
            nc.scalar.copy(out=h1T[:, :, j * P : (j + 1) * P], in_=pt3[:])

        for k in range(NT + 3):
            if k < NT:
                w2_a(k)
            if 1 <= k < NT + 1:
                w2_b(k - 1)
            if 2 <= k < NT + 2:
                s4_ln(k - 2)
            if 3 <= k:
                s4_trans(k - 3)

        p_st4.release()
        p_at.release()
        p_v.release()
        p_woh.release()

        p_w12 = tc.alloc_tile_pool(name="w12", bufs=1, side="right")
        w2_sb = p_w12.tile([P, FC, D], BF16, tag="w2")
        nc.sync.dma_start(out=w2_sb[:], in_=w2_d[:].rearrange("(k p) o -> p k o", p=P))

        # ---------- stage 5: FFN up, f1T = relu(W1^T @ h1T + b1) ----------
        p_f1 = tc.alloc_tile_pool(name="f1p", bufs=1, side="right")
        f1T = p_f1.tile([P, FC, SEQ], BF16, tag="f1T")
        def ffn1_group(n, t):
            ps = psum.tile([P, 512], F32, tag="ps", name=f"psf{n}_{t}")
            for ki in range(DC):
                nc.tensor.matmul(
                    ps[:],
                    w1_sb[:, ki, n * P : (n + 1) * P],
                    h1T[:, ki, t * 512 : (t + 1) * 512],
                    start=(ki == 0),
                    stop=(ki == DC - 1),
                )
            fslc = f1T[:, n, t * 512 : (t + 1) * 512]
            if b1_sb is not None:
                nc.vector.tensor_scalar(
                    out=fslc, in0=ps[:],
                    scalar1=b1_sb[:, n : n + 1], scalar2=0.0,
                    op0=mybir.AluOpType.add, op1=mybir.AluOpType.max,
                )
            else:
                nc.vector.tensor_scalar_max(out=fslc, in0=ps[:], scalar1=0.0)

        # ---------- stage 6: FFN down + residual + LN2 (pipelined) ----------
        def s6_main(j):
            ps = psum.tile([P, 512], F32, tag="ps", name=f"ps6_{j}")
            for n in range(FC):
                nc.tensor.matmul(
                    ps[:],
                    f1T[:, n, j * P : (j + 1) * P],
                    w2_sb[:, n, :],
                    start=(n == 0),
                    stop=(n == FC - 1),
                )
            r2 = p_f1.tile([P, D], F32, tag="r2", bufs=3, name=f"r2_{j}")
            nc.vector.tensor_add(out=r2[:], in0=h1_sb[:, j, :], in1=ps[:])
            if b2_bc is not None:
                nc.vector.tensor_add(out=r2[:], in0=r2[:], in1=b2_bc[:])
            stats = small.tile([P, 6], F32, tag="stats")
            nc.vector.bn_stats(out=stats[:], in_=r2[:])
            mv = small.tile([P, 2], F32, tag="mv")
            nc.vector.bn_aggr(out=mv[:], in_=stats[:])
            stdt = small.tile([P, 1], F32, tag="stdt")
            # stdt = sqrt((var+eps))/SX so the LN output comes out x SX
            nc.scalar.activation(
                out=stdt[:], in_=mv[:, 1:2], func=AF.Sqrt,
                bias=eps2_t[:, 0:1], scale=1.0 / (SX * SX),
            )
            rstd = small.tile([P, 1], F32, tag="rstd")
            nc.vector.reciprocal(out=rstd[:], in_=stdt[:])
            h2_t = p_f1.tile([P, D], BF16, tag="h2_t", bufs=3, name=f"h2t_{j}")
            nc.vector.tensor_scalar(
                out=h2_t[:], in0=r2[:],
                scalar1=mv[:, 0:1], scalar2=rstd[:, 0:1],
                op0=mybir.AluOpType.subtract, op1=mybir.AluOpType.mult,
            )
            if g2_bc is not None:
                nc.vector.tensor_mul(out=h2_t[:], in0=h2_t[:], in1=g2_bc[:])
            if be2_bc is not None:
                nc.vector.tensor_add(out=h2_t[:], in0=h2_t[:], in1=be2_bc[:])
            return h2_t

        h2ts = [None] * NT

        def s6_trans(j):
            pt = psum_t.tile([P, 512], BF16, tag="ptb", bufs=2, name=f"h2p{j}")
            for m in range(DC):
                nc.tensor.transpose(
                    out=pt[:, m * P : (m + 1) * P],
                    in_=h2ts[j][:, m * P : (m + 1) * P],
                    identity=ident_bf[:],
                )
            # fp8 split: hi = q8(h2T), lo = q8(h2T - hi)
            nc.scalar.copy(out=h2Th[j][:, :, :], in_=pt[:])
            nc.vector.tensor_sub(
                out=h2Tl[j][:, :, :], in0=pt[:], in1=h2Th[j][:, :, :]
            )

        # head chunks for vc=0,1 interleaved into stage-6 so PE fills LN waits
        whh_r = whh_d[:].rearrange("(k p) v -> p k v", p=P)
        whl_r = whl_d[:].rearrange("(k p) v -> p k v", p=P)

        def load_whv(vc, name):
            wh = whpool.tile([P, DC, 512], F8, tag="whv", name=f"{name}h")
            nc.sync.dma_start(out=wh[:], in_=whh_r[:, :, vc * 512 : (vc + 1) * 512])
            wl = whpool.tile([P, DC, 512], F8, tag="whv", name=f"{name}l")
            nc.sync.dma_start(out=wl[:], in_=whl_r[:, :, vc * 512 : (vc + 1) * 512])
            return wh, wl

        NWARM = 4  # head chunks interleaved into stage 6
        whvw = []
        otw = []
        for vc in range(NWARM):
            whvw.append(load_whv(vc, f"whv{vc}"))
            otw.append(opool.tile([P, NT, 512], BF16 if OUT_BF16 else F32,
                                  tag="ot", bufs=NWARM + 1, name=f"otile{vc}"))

        def head_j(whv, otile, j, toggle):
            wh, wl = whv
            ps = psum.tile([P, 512], F32, tag="ps", name=f"psh{toggle}_{j}")
            # 3-term error-corrected fp8, all DoubleRow (contract 256/instr):
            #   xh@wh + xl@wh + xh@wl
            terms = ((h2Th[j], wh), (h2Tl[j], wh), (h2Th[j], wl))
            nterm = len(terms)
            for t_i, (xt, wt) in enumerate(terms):
                for k2 in range(DC // 2):
                    nc.tensor.matmul(
                        ps[:],
                        xt[:, 2 * k2 : 2 * k2 + 2, :],
                        wt[:, 2 * k2 : 2 * k2 + 2, :],
                        start=(t_i == 0 and k2 == 0),
                        stop=(t_i == nterm - 1 and k2 == DC // 2 - 1),
                        perf_mode=DR,
                    )
            if bh_sb_for(toggle) is not None:
                sc = whpool.tile([P, 512], F32, tag="hsc", bufs=2, name=f"hsc{toggle}_{j}")
                nc.scalar.activation(
                    out=sc[:], in_=ps[:], func=AF.Identity, bias=0.0, scale=INV_SXSW,
                )
                nc.vector.tensor_add(out=otile[:, j, :], in0=sc[:], in1=bh_sb_for(toggle)[:])
            elif j % 2 == 0:
                nc.vector.tensor_scalar_mul(out=otile[:, j, :], in0=ps[:], scalar1=INV_SXSW)
            else:
                nc.scalar.activation(
                    out=otile[:, j, :], in_=ps[:], func=AF.Identity,
                    bias=0.0, scale=INV_SXSW,
                )

        _bh_tiles = {}

        def bh_sb_for(key):
            return _bh_tiles.get(key)

        if bh_d is not None:
            for vc in range(NWARM):
                bhv = whpool.tile([P, 512], F32, tag="bh", bufs=2, name=f"bh{vc}")
                nc.sync.dma_start(
                    out=bhv[:], in_=_bcast_ap(bh_d[vc * 512 : (vc + 1) * 512])
                )
                _bh_tiles[vc] = bhv

        for t in range(SEQ // 512):
            for n in range(FC):
                ffn1_group(n, t)
                if t == 1 and n % 2 == 1:
                    j = n // 2
                    h2ts[j] = s6_main(j)

        for k in range(NT + NWARM + 1):
            if 4 <= k < NT:
                h2ts[k] = s6_main(k)
            if 1 <= k <= NT:
                s6_trans(k - 1)
            for w in range(NWARM):
                if 2 + w <= k <= NT + 1 + w:
                    head_j(whvw[w], otw[w], k - 2 - w, w)
        out_rr = out_d[:].rearrange("(j p) v -> p j v", p=P)
        for vc in range(NWARM):
            nc.sync.dma_start(
                out=out_rr[:, :, vc * 512 : (vc + 1) * 512], in_=otw[vc][:]
            )

        p_f1.release()
        p_w12.release()
        p_h1.release()

        # ---------- stage 7: vocab head (vc >= 2) ----------
        out_r = out_d[:].rearrange("(j p) v -> p j v", p=P)
        for vc in range(NWARM, NV):
            whv = load_whv(vc, f"whv{vc}")
            if bh_d is not None:
                bh_bc = whpool.tile([P, 512], F32, tag="bh", bufs=2, name=f"bh{vc}")
                nc.sync.dma_start(
                    out=bh_bc[:], in_=_bcast_ap(bh_d[vc * 512 : (vc + 1) * 512])
                )
                _bh_tiles[vc] = bh_bc
            otile = opool.tile([P, NT, 512], BF16 if OUT_BF16 else F32,
                               tag="ot", bufs=NWARM + 1)
            # split stores so the final drain is short (esp. the last chunk)
            nstore = 4 if vc == NV - 1 else 2
            per = NT // nstore
            for j in range(NT):
                head_j(whv, otile, j, vc)
                if (j + 1) % per == 0:
                    nc.sync.dma_start(
                        out=out_r[:, j + 1 - per : j + 1, vc * 512 : (vc + 1) * 512],
                        in_=otile[:, j + 1 - per : j + 1, :],
                    )

        whpool.release()
        p_h2T.release()
        opool.release()
        psum_t.release()
        psum.release()
        small.release()
        const.release()

    nc.finalize()
    return nc


_PROGRAM_CACHE: dict = {}


def _get_program(flags: dict) -> bass.Bass:
    key = tuple(sorted(flags.items()))
    if key not in _PROGRAM_CACHE:
        _PROGRAM_CACHE[key] = _build_program(flags)
    return _PROGRAM_CACHE[key]


def _prep(x, embed_tab, row_embed, col_embed, Wq, bq, Wk, bk, Wv, bv, Wo, bo,
          ln1_g, ln1_b, W1, b1, W2, b2, ln2_g, ln2_b, Wh, bh):
    """Shared host-side prep: flags, common input map, per-core x shards."""
    f32c = lambda a: np.ascontiguousarray(np.asarray(a, dtype=np.float32))
    x = np.asarray(x)
    B = x.shape[0]
    assert x.shape == (B, SEQ)

    import ml_dtypes
    bfc = lambda a: np.ascontiguousarray(np.asarray(a, dtype=np.float32).astype(ml_dtypes.bfloat16))
    # fused attention weights (f32 host matmuls):
    #   scores = q k^T / sqrt(D) = h (Wq Wk^T / sqrt(D)) h^T   (biases zero)
    #   o = attn @ v @ Wo = attn @ (h (Wv Wo)) + (bv Wo + bo)
    wa = (f32c(Wq) @ f32c(Wk).T) * np.float32(1.0 / math.sqrt(D))
    wb = f32c(Wv) @ f32c(Wo)
    bc = f32c(bv) @ f32c(Wo) + f32c(bo)
    arrs = dict(
        wa=bfc(wa), wb=bfc(wb),
        w1=bfc(W1), w2=bfc(W2),
    )
    whs = f32c(Wh) * np.float32(SW)
    whh = whs.astype(ml_dtypes.float8_e4m3)
    whl = (whs - whh.astype(np.float32)).astype(ml_dtypes.float8_e4m3)
    arrs["whh"] = np.ascontiguousarray(whh)
    arrs["whl"] = np.ascontiguousarray(whl)
    pos = np.concatenate(
        [np.repeat(f32c(row_embed), GW, axis=0), np.tile(f32c(col_embed), (GH, 1))],
        axis=-1,
    ).astype(np.float32)
    arrs["maskt"] = _mask_tiles().astype(ml_dtypes.bfloat16)

    bias_map = dict(
        bc=bc, b1=f32c(b1),
        b2=f32c(b2), bh=f32c(bh),
        be1=f32c(ln1_b),
        # LN2's output is produced pre-scaled by SX; its bias must match
        be2=f32c(ln2_b) * np.float32(SX),
    )
    gain_map = dict(g1=f32c(ln1_g), g2=f32c(ln2_g))
    flags = {k: bool(np.any(v)) for k, v in bias_map.items()}
    flags.update({k: bool(np.any(v != 1.0)) for k, v in gain_map.items()})
    # the QK fusion drops per-row-constant score terms; valid only with
    # zero q/k biases (softmax shift-invariance covers the row-constant part)
    flags["bq"] = bool(np.any(f32c(bq)))
    flags["bk"] = bool(np.any(f32c(bk)))
    for k, v in {**bias_map, **gain_map}.items():
        if flags[k]:
            arrs[k] = v

    # host-side embedding gather + positional add, shipped per core in both
    # layouts: h [P, NT, D] token-major f32, ht [P, DC, SEQ] d-major bf16
    emb = f32c(embed_tab)
    hs, hts = [], []
    for c in range(B):
        h = emb[x[c]] + pos  # [SEQ, D] f32
        hs.append(np.ascontiguousarray(h.reshape(NT, P, D).transpose(1, 0, 2)))
        hts.append(np.ascontiguousarray(
            h.T.reshape(DC, P, SEQ).transpose(1, 0, 2).astype(ml_dtypes.bfloat16)
        ))
    return flags, arrs, hs, hts, B


def kernel(**inputs):
    flags, arrs, hs, hts, B = _prep(**inputs)
    nc = _get_program(flags)
    core_ids = list(range(8))
    in_maps = [{**arrs, "h": hs[c % B], "ht": hts[c % B]} for c in core_ids]
    res = run_bass_kernel_spmd(nc, in_maps, core_ids)
    out = np.stack([res.results[c]["out"] for c in range(B)], axis=0)
    return np.asarray(out, dtype=np.float32)



# revision 67
# speedup vs baseline: 1.3651x; 1.0153x over previous
"""Trainium2 Bass kernel for a small autoregressive transformer block with
local-windowed causal attention and a large (16k) vocab head.

Data-parallel over batch: batch item b runs on NeuronCore b (8 cores).
Per core:
  h   = embed_tab[x] + pos                      [1024, 512]
  q/k/v = h @ Wq/k/v (+b)                       [1024, 512]
  s   = q @ k^T / sqrt(D) + local_causal_mask   (banded, window <= 298)
  o   = softmax(s) @ v @ Wo (+bo)
  h1  = LN(h + o);  f = relu(h1@W1+b1)@W2+b2;  h2 = LN(h1 + f)
  out = h2 @ Wh (+bh)                           [1024, 16384]

All matmuls run as float32r (full-rate fp32 with N=512 moving dim).
kernel(**inputs) takes full unsharded inputs, returns [8, 1024, 16384] f32.
"""

import math
import numpy as np

import concourse.bass as bass
import concourse.mybir as mybir
import concourse.tile as tile
from concourse import bacc
from concourse.bass_utils import run_bass_kernel_spmd
from concourse.masks import make_identity

# ---- problem constants (hardcoded per contract) ----
GH = 32
GW = 32
SEQ = 1024
WIN = 9
D = 512
DFF = 1024
VOCAB = 16384
EPS = 1e-5
NEG = -1e30

P = 128
NT = SEQ // P        # 8 token chunks
DC = D // P          # 4 d chunks
FC = DFF // P        # 8 dff chunks
NV = VOCAB // 512    # 32 vocab chunks
INV_SQRT_D = 1.0 / math.sqrt(D)

F32 = mybir.dt.float32
F32R = mybir.dt.float32r
BF16 = mybir.dt.bfloat16
F8 = mybir.dt.float8e4
I32 = mybir.dt.int32
OUT_BF16 = True
AF = mybir.ActivationFunctionType
DR = mybir.MatmulPerfMode.DoubleRow

# error-corrected fp8 head: logits = xh@wh + xl@wh + xh@wl, DoubleRow matmuls.
# h2 is produced pre-scaled by SX (folded into LN2's rsqrt); Wh is pre-scaled
# by SW on the host; the eviction copy divides by SX*SW.
SX = 8.0
SW = 32.0
INV_SXSW = 1.0 / (SX * SW)


def _window_start(i: int) -> int:
    # k-window [ws, ws+512) covers all allowed keys for query chunk i
    # (max lookback is WIN*GW + WIN = 297 < 384).
    return 128 * max(0, i - 3)


def _mask_tiles() -> np.ndarray:
    idx = np.arange(SEQ)
    r, c = idx // GW, idx % GW
    allow = (
        (np.abs(r[:, None] - r[None, :]) <= WIN)
        & (np.abs(c[:, None] - c[None, :]) <= WIN)
        & (idx[None, :] <= idx[:, None])
    )
    maskf = np.where(allow, 0.0, NEG).astype(np.float32)
    tiles = np.empty((NT, P, 512), np.float32)
    for i in range(NT):
        ws = _window_start(i)
        tiles[i] = maskf[i * P : (i + 1) * P, ws : ws + 512]
    return tiles


def _r(ap):
    """bitcast to float32r for full-rate fp32 matmul."""
    return ap.bitcast(F32R)


def _bcast_ap(a: bass.AP) -> bass.AP:
    """[n] DRAM vector AP -> [P, n] partition-broadcast DMA source."""
    return bass.AP(tensor=a.tensor, offset=a.offset, ap=[[0, P], *a.ap])


def _build_program(flags: dict, wh_bufs: int = 16, msk_bufs: int = 6, lean: bool = False) -> bass.Bass:
    nc = bacc.Bacc("TRN2", target_bir_lowering=False)

    # ---------- I/O ----------
    # h = emb[x] + pos is gathered host-side and shipped in both layouts:
    # token-major (residual adds, f32) and d-major/transposed (matmul
    # operand, bf16 — quantization is well under the error budget)
    h_d = nc.declare_dram_parameter("h", [P, NT, D], F32, False)
    ht_d = nc.declare_dram_parameter("ht", [P, DC, SEQ], BF16, False)
    msk_d = nc.declare_dram_parameter("maskt", [NT, P, 512], BF16, False)
    # fused attention weights: A = Wq @ Wk^T / sqrt(D)  (scores = hA h^T),
    # B = Wv @ Wo  (o = attn @ (h B)); both computed host-side, shipped bf16.
    wa_d = nc.declare_dram_parameter("wa", [D, D], BF16, False)
    wb_d = nc.declare_dram_parameter("wb", [D, D], BF16, False)
    w1_d = nc.declare_dram_parameter("w1", [D, DFF], BF16, False)
    w2_d = nc.declare_dram_parameter("w2", [DFF, D], BF16, False)
    whh_d = nc.declare_dram_parameter("whh", [D, VOCAB], F8, False)
    whl_d = nc.declare_dram_parameter("whl", [D, VOCAB], F8, False)
    dp = lambda name, shape: nc.declare_dram_parameter(name, shape, F32, False)
    assert not (flags["bq"] or flags["bk"]), (
        "QK-fused path requires zero q/k biases (true for this problem)"
    )
    # bc = bv @ Wo + bo, folded into the attention-output residual add
    bc_d = dp("bc", [D]) if flags["bc"] else None
    b1_d = dp("b1", [DFF]) if flags["b1"] else None
    b2_d = dp("b2", [D]) if flags["b2"] else None
    bh_d = dp("bh", [VOCAB]) if flags["bh"] else None
    g1_d = dp("g1", [D]) if flags["g1"] else None
    be1_d = dp("be1", [D]) if flags["be1"] else None
    g2_d = dp("g2", [D]) if flags["g2"] else None
    be2_d = dp("be2", [D]) if flags["be2"] else None
    out_d = nc.declare_dram_parameter("out", [SEQ, VOCAB], BF16 if OUT_BF16 else F32, True)

    with tile.TileContext(nc) as tc:
        # ----- whole-kernel pools -----
        const = tc.alloc_tile_pool(name="const", bufs=1)
        small = tc.alloc_tile_pool(name="small", bufs=8)
        psum = tc.alloc_tile_pool(name="psA", bufs=6, space="PSUM")
        psum_t = tc.alloc_tile_pool(name="psT", bufs=2, space="PSUM")
        opool = tc.alloc_tile_pool(name="outev", bufs=2, side="right")
        p_h2T = tc.alloc_tile_pool(name="h2Tp", bufs=1, side="right")

        ident_f = const.tile([P, P], F32, tag="ident_f")
        eps_t = const.tile([P, 1], F32, tag="eps")
        nc.vector.memset(eps_t[:], EPS)
        # eps for LN2 with the SX scale folded in: sqrt((var+eps)/SX^2)
        eps2_t = const.tile([P, 1], F32, tag="eps2")
        nc.vector.memset(eps2_t[:], EPS / (SX * SX))

        def load_col_bias(handle, nchunks, tag):
            # [nchunks*P] DRAM -> [P, nchunks] (chunk m in column m)
            t = const.tile([P, nchunks], F32, tag=tag)
            nc.sync.dma_start(out=t[:], in_=handle[:].rearrange("(m p) -> p m", p=P))
            return t

        def load_bcast(handle, n, tag):
            t = const.tile([P, n], F32, tag=tag)
            nc.sync.dma_start(out=t[:], in_=_bcast_ap(handle[:]))
            return t

        b1_sb = load_col_bias(b1_d, FC, "b1") if b1_d else None
        bc_bc = load_bcast(bc_d, D, "bc") if bc_d else None
        b2_bc = load_bcast(b2_d, D, "b2") if b2_d else None
        g1_bc = load_bcast(g1_d, D, "g1") if g1_d else None
        be1_bc = load_bcast(be1_d, D, "be1") if be1_d else None
        g2_bc = load_bcast(g2_d, D, "g2") if g2_d else None
        be2_bc = load_bcast(be2_d, D, "be2") if be2_d else None

        h2Th = [p_h2T.tile([P, DC, P], F8, tag=f"h2Th{j}", name=f"h2Th{j}") for j in range(NT)]
        h2Tl = [p_h2T.tile([P, DC, P], F8, tag=f"h2Tl{j}", name=f"h2Tl{j}") for j in range(NT)]

        # ----- phase A pools (left, LIFO) -----
        p_woh = tc.alloc_tile_pool(name="woh", bufs=1)         # h  (-> stage 4)
        h_sb = p_woh.tile([P, NT, D], F32, tag="h")

        p_v = tc.alloc_tile_pool(name="vp", bufs=1)            # v (-> wave 2)
        v_sb = p_v.tile([P, NT, D], BF16, tag="v")
        p_at = tc.alloc_tile_pool(name="attnw", bufs=3)        # softmax work (-> stage 4)
        p_qk = tc.alloc_tile_pool(name="qkp", bufs=1)          # qAT,hT (-> wave 1)
        qT = p_qk.tile([P, DC, SEQ], BF16, tag="qT")
        hT = p_qk.tile([P, DC, SEQ], BF16, tag="hT")

        p_wq = tc.alloc_tile_pool(name="wqp", bufs=1)          # wa,wb (-> stage 2)
        wa_sb = p_wq.tile([P, DC, D], BF16, tag="wa")
        wb_sb = p_wq.tile([P, DC, D], BF16, tag="wb")

        # ---------- stage 1: load h (host-gathered emb[x]+pos) in both layouts
        # DMA order = stage-2 dependency order: wa col-chunk 0, hT half 0 by
        # ki (first matmul only needs ki=0), rest of wa, hT half 1, wb
        wa_r = wa_d[:].rearrange("(k p) o -> p k o", p=P)
        nc.sync.dma_start(out=wa_sb[:, :, 0:P], in_=wa_r[:, :, 0:P])
        for ki in range(DC):
            nc.sync.dma_start(out=hT[:, ki, 0:512], in_=ht_d[:, ki, 0:512])
        nc.sync.dma_start(out=wa_sb[:, :, P:], in_=wa_r[:, :, P:])
        for ki in range(DC):
            nc.sync.dma_start(out=hT[:, ki, 512:1024], in_=ht_d[:, ki, 512:1024])
        nc.sync.dma_start(out=wb_sb[:], in_=wb_d[:].rearrange("(k p) o -> p k o", p=P))

        make_identity(nc, ident_f[:])
        ident_bf = const.tile([P, P], BF16, tag="ident_bf")
        nc.vector.tensor_copy(out=ident_bf[:], in_=ident_f[:])

        # ---------- stage 2: qAT = (hA)^T (d-major), v = hB (token-major) ----------
        # t-major order: all groups needing hT[0:512] first (PE is in-order)
        for t in range(SEQ // 512):
            for m in range(DC):
                ps = psum.tile([P, 512], F32, tag="ps")
                for ki in range(DC):
                    nc.tensor.matmul(
                        ps[:],
                        wa_sb[:, ki, m * P : (m + 1) * P],
                        hT[:, ki, t * 512 : (t + 1) * 512],
                        start=(ki == 0),
                        stop=(ki == DC - 1),
                    )
                dslc = qT[:, m, t * 512 : (t + 1) * 512]
                if m % 2 == 0:
                    nc.vector.tensor_copy(out=dslc, in_=ps[:])
                else:
                    nc.scalar.copy(out=dslc, in_=ps[:])
            for j in range(4 * t, 4 * t + 4):
                ps = psum.tile([P, 512], F32, tag="ps")
                for ki in range(DC):
                    nc.tensor.matmul(
                        ps[:],
                        hT[:, ki, j * P : (j + 1) * P],
                        wb_sb[:, ki, :],
                        start=(ki == 0),
                        stop=(ki == DC - 1),
                    )
                if j % 2 == 0:
                    nc.scalar.copy(out=v_sb[:, j, :], in_=ps[:])
                else:
                    nc.vector.tensor_copy(out=v_sb[:, j, :], in_=ps[:])

        p_wq.release()

        # ---------- stage 3 wave 1: scores + softmax ----------
        # all mask loads up front, then the deferred token-major h load
        msk_ts = []
        for i in range(NT):
            msk_t = p_at.tile([P, 512], BF16, tag="msk", bufs=NT, name=f"msk{i}")
            nc.sync.dma_start(out=msk_t[:], in_=msk_d[i])
            msk_ts.append(msk_t)
        nc.sync.dma_start(out=h_sb[:], in_=h_d[:])

        attns = []
        recips = []
        for i in range(NT):
            ws = _window_start(i)
            nw = min(512, max(256, (i + 1) * P))  # live window
            msk_t = msk_ts[i]
            ps_s = psum.tile([P, 512], F32, tag="ps")
            for ki in range(DC):
                nc.tensor.matmul(
                    ps_s[:, :nw],
                    qT[:, ki, i * P : (i + 1) * P],
                    hT[:, ki, ws : ws + nw],
                    start=(ki == 0),
                    stop=False,
                )
            # additive mask folded into the PSUM group: ps += I^T @ mask
            nc.tensor.matmul(
                ps_s[:, :nw], ident_bf[:], msk_t[:, :nw], start=False, stop=True,
            )
            attn = p_at.tile([P, 512], BF16, tag="attn", bufs=NT, name=f"attn{i}")
            denom = small.tile([P, 1], F32, tag="denom")
            # A carries the 1/sqrt(D) factor, so the scores arrive pre-scaled
            nc.scalar.activation(
                out=attn[:, :nw], in_=ps_s[:, :nw], func=AF.Exp,
                bias=0.0, scale=1.0,
                accum_out=denom[:, 0:1],
            )
            recip = small.tile([P, 1], F32, tag="recip", bufs=NT, name=f"recip{i}")
            nc.vector.reciprocal(out=recip[:], in_=denom[:])
            attns.append(attn)
            recips.append(recip)

        p_qk.release()

        # ----- right-side pools for FFN phase -----
        whpool = tc.alloc_tile_pool(name="whstream", bufs=wh_bufs, side="right")
        p_h1 = tc.alloc_tile_pool(name="h1p", bufs=1, side="right")
        h1_sb = p_h1.tile([P, NT, D], BF16, tag="h1")
        h1T = p_h1.tile([P, DC, SEQ], BF16, tag="h1T")
        w1_sb = p_h1.tile([P, DC, DFF], BF16, tag="w1")
        nc.sync.dma_start(out=w1_sb[:], in_=w1_d[:].rearrange("(k p) o -> p k o", p=P))

        # ---------- stage 3 wave 2 + stage 4, software-pipelined ----------
        p_st4 = tc.alloc_tile_pool(name="st4", bufs=3)
        attnTs = [None] * NT
        o_ps = [None] * NT

        def w2_a(i):  # attn transposes (bf16) + attnT eviction
            ws = _window_start(i)
            kb0 = ws // P
            nkb = min(DC, i - kb0 + 1)
            pt = psum_t.tile([P, 512], BF16, tag="ptb", bufs=4, name=f"atp{i}")
            for kk in range(nkb):
                nc.tensor.transpose(
                    out=pt[:, kk * P : (kk + 1) * P],
                    in_=attns[i][:, kk * P : (kk + 1) * P],
                    identity=ident_bf[:],
                )
            attnT = p_at.tile([P, 512], BF16, tag="attnT", bufs=3, name=f"attnT{i}")
            nc.scalar.copy(out=attnT[:, : nkb * P], in_=pt[:, : nkb * P])
            attnTs[i] = attnT

        def w2_b(i):  # o matmuls + scale
            ws = _window_start(i)
            kb0 = ws // P
            nkb = min(DC, i - kb0 + 1)
            ps_o = psum.tile([P, 512], F32, tag="ps", name=f"pso{i}")
            for kk in range(nkb):
                nc.tensor.matmul(
                    ps_o[:],
                    attnTs[i][:, kk * P : (kk + 1) * P],
                    v_sb[:, kb0 + kk, :],
                    start=(kk == 0),
                    stop=(kk == nkb - 1),
                )
            o_ps[i] = ps_o

        def s4_ln(j):  # residual + LN1 (o is already fully projected via B)
            r1 = p_st4.tile([P, D], F32, tag="r1", name=f"r1_{j}")
            # fused softmax-normalize + residual: r1 = o_psum * recip + h
            nc.vector.scalar_tensor_tensor(
                out=r1[:], in0=o_ps[j][:], scalar=recips[j][:, 0:1],
                in1=h_sb[:, j, :],
                op0=mybir.AluOpType.mult, op1=mybir.AluOpType.add,
            )
            if bc_bc is not None:
                nc.vector.tensor_add(out=r1[:], in0=r1[:], in1=bc_bc[:])
            stats = small.tile([P, 6], F32, tag="stats")
            nc.vector.bn_stats(out=stats[:], in_=r1[:])
            mv = small.tile([P, 2], F32, tag="mv")
            nc.vector.bn_aggr(out=mv[:], in_=stats[:])
            stdt = small.tile([P, 1], F32, tag="stdt")
            nc.scalar.activation(
                out=stdt[:], in_=mv[:, 1:2], func=AF.Sqrt,
                bias=eps_t[:, 0:1], scale=1.0,
            )
            rstd = small.tile([P, 1], F32, tag="rstd")
            nc.vector.reciprocal(out=rstd[:], in_=stdt[:])
            nc.vector.tensor_scalar(
                out=h1_sb[:, j, :], in0=r1[:],
                scalar1=mv[:, 0:1], scalar2=rstd[:, 0:1],
                op0=mybir.AluOpType.subtract, op1=mybir.AluOpType.mult,
            )
            if g1_bc is not None:
                nc.vector.tensor_mul(out=h1_sb[:, j, :], in0=h1_sb[:, j, :], in1=g1_bc[:])
            if be1_bc is not None:
                nc.vector.tensor_add(out=h1_sb[:, j, :], in0=h1_sb[:, j, :], in1=be1_bc[:])

        def s4_trans(j):  # h1 transposes (bf16) + h1T eviction
            pt3 = psum_t.tile([P, 512], BF16, tag="ptb", bufs=4, name=f"h1p{j}")
            for m in range(DC):
                nc.tensor.transpose(
                    out=pt3[:, m * P : (m + 1) * P],
                    in_=h1_sb[:, j, m * P : (m + 1) * P],
                    identity=ident_bf[:],
                )
            nc.scalar.copy(out=h1T[:, :, j * P : (j + 1) * P], in_=pt3[:])

        for k in range(NT + 3):
            if k < NT:
                w2_a(k)
            if 1 <= k < NT + 1:
                w2_b(k - 1)
            if 2 <= k < NT + 2:
                s4_ln(k - 2)
            if 3 <= k:
                s4_trans(k - 3)

        p_st4.release()
        p_at.release()
        p_v.release()
        p_woh.release()

        p_w12 = tc.alloc_tile_pool(name="w12", bufs=1, side="right")
        w2_sb = p_w12.tile([P, FC, D], BF16, tag="w2")
        nc.sync.dma_start(out=w2_sb[:], in_=w2_d[:].rearrange("(k p) o -> p k o", p=P))

        # ---------- stage 5: FFN up, f1T = relu(W1^T @ h1T + b1) ----------
        p_f1 = tc.alloc_tile_pool(name="f1p", bufs=1, side="right")
        f1T = p_f1.tile([P, FC, SEQ], BF16, tag="f1T")
        def ffn1_group(n, t):
            ps = psum.tile([P, 512], F32, tag="ps", name=f"psf{n}_{t}")
            for ki in range(DC):
                nc.tensor.matmul(
                    ps[:],
                    w1_sb[:, ki, n * P : (n + 1) * P],
                    h1T[:, ki, t * 512 : (t + 1) * 512],
                    start=(ki == 0),
                    stop=(ki == DC - 1),
                )
            fslc = f1T[:, n, t * 512 : (t + 1) * 512]
            if b1_sb is not None:
                nc.scalar.activation(
                    out=fslc, in_=ps[:], func=AF.Relu,
                    bias=b1_sb[:, n : n + 1], scale=1.0,
                )
            elif n % 2 == 0:
                nc.vector.tensor_scalar_max(out=fslc, in0=ps[:], scalar1=0.0)
            else:
                nc.scalar.activation(
                    out=fslc, in_=ps[:], func=AF.Relu, bias=0.0, scale=1.0,
                )

        # ---------- stage 6: FFN down + residual + LN2 (pipelined) ----------
        def s6_main(j):
            ps = psum.tile([P, 512], F32, tag="ps", name=f"ps6_{j}")
            for n in range(FC):
                nc.tensor.matmul(
                    ps[:],
                    f1T[:, n, j * P : (j + 1) * P],
                    w2_sb[:, n, :],
                    start=(n == 0),
                    stop=(n == FC - 1),
                )
            r2 = p_f1.tile([P, D], F32, tag="r2", bufs=3, name=f"r2_{j}")
            nc.vector.tensor_add(out=r2[:], in0=h1_sb[:, j, :], in1=ps[:])
            if b2_bc is not None:
                nc.vector.tensor_add(out=r2[:], in0=r2[:], in1=b2_bc[:])
            stats = small.tile([P, 6], F32, tag="stats")
            nc.vector.bn_stats(out=stats[:], in_=r2[:])
            mv = small.tile([P, 2], F32, tag="mv")
            nc.vector.bn_aggr(out=mv[:], in_=stats[:])
            stdt = small.tile([P, 1], F32, tag="stdt")
            # stdt = sqrt((var+eps))/SX so the LN output comes out x SX
            nc.scalar.activation(
                out=stdt[:], in_=mv[:, 1:2], func=AF.Sqrt,
                bias=eps2_t[:, 0:1], scale=1.0 / (SX * SX),
            )
            rstd = small.tile([P, 1], F32, tag="rstd")
            nc.vector.reciprocal(out=rstd[:], in_=stdt[:])
            h2_t = p_f1.tile([P, D], BF16, tag="h2_t", bufs=3, name=f"h2t_{j}")
            nc.vector.tensor_scalar(
                out=h2_t[:], in0=r2[:],
                scalar1=mv[:, 0:1], scalar2=rstd[:, 0:1],
                op0=mybir.AluOpType.subtract, op1=mybir.AluOpType.mult,
            )
            if g2_bc is not None:
                nc.vector.tensor_mul(out=h2_t[:], in0=h2_t[:], in1=g2_bc[:])
            if be2_bc is not None:
                nc.vector.tensor_add(out=h2_t[:], in0=h2_t[:], in1=be2_bc[:])
            return h2_t

        h2ts = [None] * NT

        def s6_trans(j):
            pt = psum_t.tile([P, 512], BF16, tag="ptb", bufs=4, name=f"h2p{j}")
            for m in range(DC):
                nc.tensor.transpose(
                    out=pt[:, m * P : (m + 1) * P],
                    in_=h2ts[j][:, m * P : (m + 1) * P],
                    identity=ident_bf[:],
                )
            # fp8 split: hi = q8(h2T), lo = q8(h2T - hi)
            nc.scalar.copy(out=h2Th[j][:, :, :], in_=pt[:])
            nc.vector.tensor_sub(
                out=h2Tl[j][:, :, :], in0=pt[:], in1=h2Th[j][:, :, :]
            )

        # head chunks for vc=0,1 interleaved into stage-6 so PE fills LN waits
        whh_r = whh_d[:].rearrange("(k p) v -> p k v", p=P)
        whl_r = whl_d[:].rearrange("(k p) v -> p k v", p=P)

        def load_whv(vc, name):
            wh = whpool.tile([P, DC, 512], F8, tag="whv", name=f"{name}h")
            nc.sync.dma_start(out=wh[:], in_=whh_r[:, :, vc * 512 : (vc + 1) * 512])
            wl = whpool.tile([P, DC, 512], F8, tag="whv", name=f"{name}l")
            nc.sync.dma_start(out=wl[:], in_=whl_r[:, :, vc * 512 : (vc + 1) * 512])
            return wh, wl

        NWARM = 4  # head chunks interleaved into stage 6
        whvw = []
        otw = []
        for vc in range(NWARM):
            whvw.append(load_whv(vc, f"whv{vc}"))
            otw.append(opool.tile([P, NT, 512], BF16 if OUT_BF16 else F32,
                                  tag="ot", bufs=NWARM + 1, name=f"otile{vc}"))

        def head_j(whv, otile, j, toggle):
            wh, wl = whv
            ps = psum.tile([P, 512], F32, tag="ps", name=f"psh{toggle}_{j}")
            # 3-term error-corrected fp8, all DoubleRow (contract 256/instr):
            #   xh@wh + xl@wh + xh@wl
            terms = ((h2Th[j], wh), (h2Tl[j], wh), (h2Th[j], wl))
            nterm = len(terms)
            for t_i, (xt, wt) in enumerate(terms):
                for k2 in range(DC // 2):
                    nc.tensor.matmul(
                        ps[:],
                        xt[:, 2 * k2 : 2 * k2 + 2, :],
                        wt[:, 2 * k2 : 2 * k2 + 2, :],
                        start=(t_i == 0 and k2 == 0),
                        stop=(t_i == nterm - 1 and k2 == DC // 2 - 1),
                        perf_mode=DR,
                    )
            if bh_sb_for(toggle) is not None:
                sc = whpool.tile([P, 512], F32, tag="hsc", bufs=2, name=f"hsc{toggle}_{j}")
                nc.scalar.activation(
                    out=sc[:], in_=ps[:], func=AF.Identity, bias=0.0, scale=INV_SXSW,
                )
                nc.vector.tensor_add(out=otile[:, j, :], in0=sc[:], in1=bh_sb_for(toggle)[:])
            elif j % 2 == 0:
                nc.vector.tensor_scalar_mul(out=otile[:, j, :], in0=ps[:], scalar1=INV_SXSW)
            else:
                nc.scalar.activation(
                    out=otile[:, j, :], in_=ps[:], func=AF.Identity,
                    bias=0.0, scale=INV_SXSW,
                )

        _bh_tiles = {}

        def bh_sb_for(key):
            return _bh_tiles.get(key)

        if bh_d is not None:
            for vc in range(NWARM):
                bhv = whpool.tile([P, 512], F32, tag="bh", bufs=2, name=f"bh{vc}")
                nc.sync.dma_start(
                    out=bhv[:], in_=_bcast_ap(bh_d[vc * 512 : (vc + 1) * 512])
                )
                _bh_tiles[vc] = bhv

        for t in range(SEQ // 512):
            for n in range(FC):
                ffn1_group(n, t)
                if t == 1 and n % 2 == 1:
                    j = n // 2
                    h2ts[j] = s6_main(j)

        for k in range(NT + NWARM + 1):
            if 4 <= k < NT:
                h2ts[k] = s6_main(k)
            if 1 <= k <= NT:
                s6_trans(k - 1)
            for w in range(NWARM):
                if 2 + w <= k <= NT + 1 + w:
                    head_j(whvw[w], otw[w], k - 2 - w, w)
        out_rr = out_d[:].rearrange("(j p) v -> p j v", p=P)
        for vc in range(NWARM):
            nc.sync.dma_start(
                out=out_rr[:, :, vc * 512 : (vc + 1) * 512], in_=otw[vc][:]
            )

        p_f1.release()
        p_w12.release()
        p_h1.release()

        # ---------- stage 7: vocab head (vc >= 2) ----------
        out_r = out_d[:].rearrange("(j p) v -> p j v", p=P)
        for vc in range(NWARM, NV):
            whv = load_whv(vc, f"whv{vc}")
            if bh_d is not None:
                bh_bc = whpool.tile([P, 512], F32, tag="bh", bufs=2, name=f"bh{vc}")
                nc.sync.dma_start(
                    out=bh_bc[:], in_=_bcast_ap(bh_d[vc * 512 : (vc + 1) * 512])
                )
                _bh_tiles[vc] = bh_bc
            otile = opool.tile([P, NT, 512], BF16 if OUT_BF16 else F32,
                               tag="ot", bufs=NWARM + 1)
            # split stores so the final drain is short (esp. the last chunk)
            nstore = 4 if vc == NV - 1 else 2
            per = NT // nstore
            for j in range(NT):
                head_j(whv, otile, j, vc)
                if (j + 1) % per == 0:
                    nc.sync.dma_start(
                        out=out_r[:, j + 1 - per : j + 1, vc * 512 : (vc + 1) * 512],
                        in_=otile[:, j + 1 - per : j + 1, :],
                    )

        whpool.release()
        p_h2T.release()
        opool.release()
        psum_t.release()
        psum.release()
        small.release()
        const.release()

    nc.finalize()
    return nc


_PROGRAM_CACHE: dict = {}


def _get_program(flags: dict) -> bass.Bass:
    key = tuple(sorted(flags.items()))
    if key not in _PROGRAM_CACHE:
        _PROGRAM_CACHE[key] = _build_program(flags)
    return _PROGRAM_CACHE[key]


def _prep(x, embed_tab, row_embed, col_embed, Wq, bq, Wk, bk, Wv, bv, Wo, bo,
          ln1_g, ln1_b, W1, b1, W2, b2, ln2_g, ln2_b, Wh, bh):
    """Shared host-side prep: flags, common input map, per-core x shards."""
    f32c = lambda a: np.ascontiguousarray(np.asarray(a, dtype=np.float32))
    x = np.asarray(x)
    B = x.shape[0]
    assert x.shape == (B, SEQ)

    import ml_dtypes
    bfc = lambda a: np.ascontiguousarray(np.asarray(a, dtype=np.float32).astype(ml_dtypes.bfloat16))
    # fused attention weights (f32 host matmuls):
    #   scores = q k^T / sqrt(D) = h (Wq Wk^T / sqrt(D)) h^T   (biases zero)
    #   o = attn @ v @ Wo = attn @ (h (Wv Wo)) + (bv Wo + bo)
    wa = (f32c(Wq) @ f32c(Wk).T) * np.float32(1.0 / math.sqrt(D))
    wb = f32c(Wv) @ f32c(Wo)
    bc = f32c(bv) @ f32c(Wo) + f32c(bo)
    arrs = dict(
        wa=bfc(wa), wb=bfc(wb),
        w1=bfc(W1), w2=bfc(W2),
    )
    whs = f32c(Wh) * np.float32(SW)
    whh = whs.astype(ml_dtypes.float8_e4m3)
    whl = (whs - whh.astype(np.float32)).astype(ml_dtypes.float8_e4m3)
    arrs["whh"] = np.ascontiguousarray(whh)
    arrs["whl"] = np.ascontiguousarray(whl)
    pos = np.concatenate(
        [np.repeat(f32c(row_embed), GW, axis=0), np.tile(f32c(col_embed), (GH, 1))],
        axis=-1,
    ).astype(np.float32)
    arrs["maskt"] = _mask_tiles().astype(ml_dtypes.bfloat16)

    bias_map = dict(
        bc=bc, b1=f32c(b1),
        b2=f32c(b2), bh=f32c(bh),
        be1=f32c(ln1_b),
        # LN2's output is produced pre-scaled by SX; its bias must match
        be2=f32c(ln2_b) * np.float32(SX),
    )
    gain_map = dict(g1=f32c(ln1_g), g2=f32c(ln2_g))
    flags = {k: bool(np.any(v)) for k, v in bias_map.items()}
    flags.update({k: bool(np.any(v != 1.0)) for k, v in gain_map.items()})
    # the QK fusion drops per-row-constant score terms; valid only with
    # zero q/k biases (softmax shift-invariance covers the row-constant part)
    flags["bq"] = bool(np.any(f32c(bq)))
    flags["bk"] = bool(np.any(f32c(bk)))
    for k, v in {**bias_map, **gain_map}.items():
        if flags[k]:
            arrs[k] = v

    # host-side embedding gather + positional add, shipped per core in both
    # layouts: h [P, NT, D] token-major f32, ht [P, DC, SEQ] d-major bf16
    emb = f32c(embed_tab)
    hs, hts = [], []
    for c in range(B):
        h = emb[x[c]] + pos  # [SEQ, D] f32
        hs.append(np.ascontiguousarray(h.reshape(NT, P, D).transpose(1, 0, 2)))
        hts.append(np.ascontiguousarray(
            h.T.reshape(DC, P, SEQ).transpose(1, 0, 2).astype(ml_dtypes.bfloat16)
        ))
    return flags, arrs, hs, hts, B


def kernel(**inputs):
    flags, arrs, hs, hts, B = _prep(**inputs)
    nc = _get_program(flags)
    core_ids = list(range(8))
    in_maps = [{**arrs, "h": hs[c % B], "ht": hts[c % B]} for c in core_ids]
    res = run_bass_kernel_spmd(nc, in_maps, core_ids)
    out = np.stack([res.results[c]["out"] for c in range(B)], axis=0)
    return np.asarray(out, dtype=np.float32)



# revision 81
# speedup vs baseline: 1.3715x; 1.0047x over previous
"""Trainium2 Bass kernel for a small autoregressive transformer block with
local-windowed causal attention and a large (16k) vocab head.

Data-parallel over batch: batch item b runs on NeuronCore b (8 cores).
Per core:
  h   = embed_tab[x] + pos                      [1024, 512]
  q/k/v = h @ Wq/k/v (+b)                       [1024, 512]
  s   = q @ k^T / sqrt(D) + local_causal_mask   (banded, window <= 298)
  o   = softmax(s) @ v @ Wo (+bo)
  h1  = LN(h + o);  f = relu(h1@W1+b1)@W2+b2;  h2 = LN(h1 + f)
  out = h2 @ Wh (+bh)                           [1024, 16384]

All matmuls run as float32r (full-rate fp32 with N=512 moving dim).
kernel(**inputs) takes full unsharded inputs, returns [8, 1024, 16384] f32.
"""

import math
import numpy as np

import concourse.bass as bass
import concourse.mybir as mybir
import concourse.tile as tile
from concourse import bacc
from concourse.bass_utils import run_bass_kernel_spmd
from concourse.masks import make_identity

# ---- problem constants (hardcoded per contract) ----
GH = 32
GW = 32
SEQ = 1024
WIN = 9
D = 512
DFF = 1024
VOCAB = 16384
EPS = 1e-5
NEG = -240.0  # exact in fp8-e4m3; exp(s + NEG) underflows to 0

P = 128
NT = SEQ // P        # 8 token chunks
DC = D // P          # 4 d chunks
FC = DFF // P        # 8 dff chunks
NV = VOCAB // 512    # 32 vocab chunks
INV_SQRT_D = 1.0 / math.sqrt(D)

F32 = mybir.dt.float32
F32R = mybir.dt.float32r
BF16 = mybir.dt.bfloat16
F8 = mybir.dt.float8e4
I32 = mybir.dt.int32
OUT_BF16 = True
AF = mybir.ActivationFunctionType
DR = mybir.MatmulPerfMode.DoubleRow

# error-corrected fp8 head: logits = xh@wh + xl@wh + xh@wl, DoubleRow matmuls.
# h2 is produced pre-scaled by SX (folded into LN2's rsqrt); Wh is pre-scaled
# by SW on the host; the eviction copy divides by SX*SW.
SX = 8.0
SW = 32.0
INV_SXSW = 1.0 / (SX * SW)


def _window_start(i: int) -> int:
    # k-window [ws, ws+512) covers all allowed keys for query chunk i
    # (max lookback is WIN*GW + WIN = 297 < 384).
    return 128 * max(0, i - 3)


def _mask_tiles() -> np.ndarray:
    idx = np.arange(SEQ)
    r, c = idx // GW, idx % GW
    allow = (
        (np.abs(r[:, None] - r[None, :]) <= WIN)
        & (np.abs(c[:, None] - c[None, :]) <= WIN)
        & (idx[None, :] <= idx[:, None])
    )
    maskf = np.where(allow, 0.0, NEG).astype(np.float32)
    tiles = np.empty((NT, P, 512), np.float32)
    for i in range(NT):
        ws = _window_start(i)
        tiles[i] = maskf[i * P : (i + 1) * P, ws : ws + 512]
    return tiles


def _r(ap):
    """bitcast to float32r for full-rate fp32 matmul."""
    return ap.bitcast(F32R)


def _bcast_ap(a: bass.AP) -> bass.AP:
    """[n] DRAM vector AP -> [P, n] partition-broadcast DMA source."""
    return bass.AP(tensor=a.tensor, offset=a.offset, ap=[[0, P], *a.ap])


def _build_program(flags: dict, wh_bufs: int = 16, msk_bufs: int = 6, lean: bool = False) -> bass.Bass:
    nc = bacc.Bacc("TRN2", target_bir_lowering=False)

    # ---------- I/O ----------
    # h = emb[x] + pos is gathered host-side and shipped in both layouts:
    # token-major (residual adds, f32) and d-major/transposed (matmul
    # operand, bf16 — quantization is well under the error budget)
    h_d = nc.declare_dram_parameter("h", [P, NT, D], F32, False)
    ht_d = nc.declare_dram_parameter("ht", [P, DC, SEQ], BF16, False)
    msk_d = nc.declare_dram_parameter("maskt", [NT, P, 512], F8, False)
    # fused attention weights: A = Wq @ Wk^T / sqrt(D)  (scores = hA h^T),
    # B = Wv @ Wo  (o = attn @ (h B)); both computed host-side, shipped bf16.
    wa_d = nc.declare_dram_parameter("wa", [D, D], BF16, False)
    wb_d = nc.declare_dram_parameter("wb", [D, D], BF16, False)
    w1_d = nc.declare_dram_parameter("w1", [D, DFF], BF16, False)
    w2_d = nc.declare_dram_parameter("w2", [DFF, D], BF16, False)
    whh_d = nc.declare_dram_parameter("whh", [D, VOCAB], F8, False)
    whl_d = nc.declare_dram_parameter("whl", [D, VOCAB], F8, False)
    dp = lambda name, shape: nc.declare_dram_parameter(name, shape, F32, False)
    assert not (flags["bq"] or flags["bk"]), (
        "QK-fused path requires zero q/k biases (true for this problem)"
    )
    # bc = bv @ Wo + bo, folded into the attention-output residual add
    bc_d = dp("bc", [D]) if flags["bc"] else None
    b1_d = dp("b1", [DFF]) if flags["b1"] else None
    b2_d = dp("b2", [D]) if flags["b2"] else None
    bh_d = dp("bh", [VOCAB]) if flags["bh"] else None
    g1_d = dp("g1", [D]) if flags["g1"] else None
    be1_d = dp("be1", [D]) if flags["be1"] else None
    g2_d = dp("g2", [D]) if flags["g2"] else None
    be2_d = dp("be2", [D]) if flags["be2"] else None
    out_d = nc.declare_dram_parameter("out", [SEQ, VOCAB], BF16 if OUT_BF16 else F32, True)

    with tile.TileContext(nc) as tc:
        # ----- whole-kernel pools -----
        const = tc.alloc_tile_pool(name="const", bufs=1)
        small = tc.alloc_tile_pool(name="small", bufs=8)
        psum = tc.alloc_tile_pool(name="psA", bufs=5, space="PSUM")
        psum_t = tc.alloc_tile_pool(name="psT", bufs=2, space="PSUM")
        opool = tc.alloc_tile_pool(name="outev", bufs=2, side="right")
        p_h2T = tc.alloc_tile_pool(name="h2Tp", bufs=1, side="right")

        ident_f = const.tile([P, P], F32, tag="ident_f")
        eps_t = const.tile([P, 1], F32, tag="eps")
        nc.vector.memset(eps_t[:], EPS)
        # eps for LN2 with the SX scale folded in: sqrt((var+eps)/SX^2)
        eps2_t = const.tile([P, 1], F32, tag="eps2")
        nc.vector.memset(eps2_t[:], EPS / (SX * SX))

        def load_col_bias(handle, nchunks, tag):
            # [nchunks*P] DRAM -> [P, nchunks] (chunk m in column m)
            t = const.tile([P, nchunks], F32, tag=tag)
            nc.sync.dma_start(out=t[:], in_=handle[:].rearrange("(m p) -> p m", p=P))
            return t

        def load_bcast(handle, n, tag):
            t = const.tile([P, n], F32, tag=tag)
            nc.sync.dma_start(out=t[:], in_=_bcast_ap(handle[:]))
            return t

        b1_sb = load_col_bias(b1_d, FC, "b1") if b1_d else None
        bc_bc = load_bcast(bc_d, D, "bc") if bc_d else None
        b2_bc = load_bcast(b2_d, D, "b2") if b2_d else None
        g1_bc = load_bcast(g1_d, D, "g1") if g1_d else None
        be1_bc = load_bcast(be1_d, D, "be1") if be1_d else None
        g2_bc = load_bcast(g2_d, D, "g2") if g2_d else None
        be2_bc = load_bcast(be2_d, D, "be2") if be2_d else None

        h2Th = [p_h2T.tile([P, DC, P], F8, tag=f"h2Th{j}", name=f"h2Th{j}") for j in range(NT)]
        h2Tl = [p_h2T.tile([P, DC, P], F8, tag=f"h2Tl{j}", name=f"h2Tl{j}") for j in range(NT)]

        # ----- phase A pools (left, LIFO) -----
        p_woh = tc.alloc_tile_pool(name="woh", bufs=1)         # h  (-> stage 4)
        h_sb = p_woh.tile([P, NT, D], F32, tag="h")

        p_v = tc.alloc_tile_pool(name="vp", bufs=1)            # v (-> wave 2)
        v_sb = p_v.tile([P, NT, D], BF16, tag="v")
        p_at = tc.alloc_tile_pool(name="attnw", bufs=3)        # softmax work (-> stage 4)
        p_qk = tc.alloc_tile_pool(name="qkp", bufs=1)          # qAT,hT (-> wave 1)
        qT = p_qk.tile([P, DC, SEQ], BF16, tag="qT")
        hT = p_qk.tile([P, DC, SEQ], BF16, tag="hT")

        p_wq = tc.alloc_tile_pool(name="wqp", bufs=1)          # wa,wb (-> stage 2)
        wa_sb = p_wq.tile([P, DC, D], BF16, tag="wa")
        wb_sb = p_wq.tile([P, DC, D], BF16, tag="wb")

        # ---------- stage 1: load h (host-gathered emb[x]+pos) in both layouts
        # DMA order = stage-2 dependency order: wa col-chunk 0, hT half 0 by
        # ki (first matmul only needs ki=0), rest of wa, hT half 1, wb
        wa_r = wa_d[:].rearrange("(k p) o -> p k o", p=P)
        nc.sync.dma_start(out=wa_sb[:, :, 0:P], in_=wa_r[:, :, 0:P])
        for ki in range(DC):
            nc.sync.dma_start(out=hT[:, ki, 0:512], in_=ht_d[:, ki, 0:512])
        nc.sync.dma_start(out=wa_sb[:, :, P : 2 * P], in_=wa_r[:, :, P : 2 * P])
        # wb before hT half 1: the first v groups only touch hT[:, :, 0:512]
        nc.sync.dma_start(out=wb_sb[:], in_=wb_d[:].rearrange("(k p) o -> p k o", p=P))
        nc.sync.dma_start(out=wa_sb[:, :, 2 * P :], in_=wa_r[:, :, 2 * P :])
        for ki in range(DC):
            nc.sync.dma_start(out=hT[:, ki, 512:1024], in_=ht_d[:, ki, 512:1024])

        make_identity(nc, ident_f[:])
        ident_bf = const.tile([P, P], BF16, tag="ident_bf")
        nc.vector.tensor_copy(out=ident_bf[:], in_=ident_f[:])
        ident_f8 = const.tile([P, P], F8, tag="ident_f8")
        nc.vector.tensor_copy(out=ident_f8[:], in_=ident_f[:])

        # ---------- stage 2 + wave 1 merged: qAT, scores/softmax, v ----------
        # all mask loads up front, then the deferred token-major h load
        msk_ts = []
        for i in range(NT):
            msk_t = p_at.tile([P, 512], F8, tag="msk", bufs=NT, name=f"msk{i}")
            nc.sync.dma_start(out=msk_t[:], in_=msk_d[i])
            msk_ts.append(msk_t)
        nc.sync.dma_start(out=h_sb[:], in_=h_d[:])

        attns = [None] * NT
        recips = [None] * NT

        def scores_i(i):
            ws = _window_start(i)
            nw = min(512, (i + 1) * P)  # live window (bf16: any size ok)
            ps_s = psum.tile([P, 512], F32, tag="ps")
            for ki in range(DC):
                nc.tensor.matmul(
                    ps_s[:, :nw],
                    qT[:, ki, i * P : (i + 1) * P],
                    hT[:, ki, ws : ws + nw],
                    start=(ki == 0),
                    stop=False,
                )
            # additive mask folded into the PSUM group: ps += I^T @ mask
            nc.tensor.matmul(
                ps_s[:, :nw], ident_f8[:], msk_ts[i][:, :nw], start=False, stop=True,
            )
            attn = p_at.tile([P, 512], BF16, tag="attn", bufs=NT, name=f"attn{i}")
            denom = small.tile([P, 1], F32, tag="denom")
            # A carries the 1/sqrt(D) factor, so the scores arrive pre-scaled
            nc.scalar.activation(
                out=attn[:, :nw], in_=ps_s[:, :nw], func=AF.Exp,
                bias=0.0, scale=1.0,
                accum_out=denom[:, 0:1],
            )
            recip = small.tile([P, 1], F32, tag="recip", bufs=NT, name=f"recip{i}")
            nc.vector.reciprocal(out=recip[:], in_=denom[:])
            attns[i] = attn
            recips[i] = recip

        # t-major order: all groups needing hT[0:512] first (PE is in-order);
        # scores for the finished t-half run among the v groups so the
        # softmax chain (ACT/DVE) hides behind stage-2 PE work
        for t in range(SEQ // 512):
            for m in range(DC):
                ps = psum.tile([P, 512], F32, tag="ps")
                for ki in range(DC):
                    nc.tensor.matmul(
                        ps[:],
                        wa_sb[:, ki, m * P : (m + 1) * P],
                        hT[:, ki, t * 512 : (t + 1) * 512],
                        start=(ki == 0),
                        stop=(ki == DC - 1),
                    )
                dslc = qT[:, m, t * 512 : (t + 1) * 512]
                if m % 2 == 0:
                    nc.vector.tensor_copy(out=dslc, in_=ps[:])
                else:
                    nc.scalar.copy(out=dslc, in_=ps[:])
            for j in range(4 * t, 4 * t + 4):
                scores_i(j)
                ps = psum.tile([P, 512], F32, tag="ps")
                for ki in range(DC):
                    nc.tensor.matmul(
                        ps[:],
                        hT[:, ki, j * P : (j + 1) * P],
                        wb_sb[:, ki, :],
                        start=(ki == 0),
                        stop=(ki == DC - 1),
                    )
                if j % 2 == 0:
                    nc.scalar.copy(out=v_sb[:, j, :], in_=ps[:])
                else:
                    nc.vector.tensor_copy(out=v_sb[:, j, :], in_=ps[:])

        p_wq.release()
        p_qk.release()

        # ----- right-side pools for FFN phase -----
        whpool = tc.alloc_tile_pool(name="whstream", bufs=wh_bufs, side="right")
        p_h1 = tc.alloc_tile_pool(name="h1p", bufs=1, side="right")
        h1_sb = p_h1.tile([P, NT, D], BF16, tag="h1")
        h1T = p_h1.tile([P, DC, SEQ], BF16, tag="h1T")
        w1_sb = p_h1.tile([P, DC, DFF], BF16, tag="w1")
        nc.sync.dma_start(out=w1_sb[:], in_=w1_d[:].rearrange("(k p) o -> p k o", p=P))

        # ---------- stage 3 wave 2 + stage 4, software-pipelined ----------
        p_st4 = tc.alloc_tile_pool(name="st4", bufs=3)
        attnTs = [None] * NT
        o_ps = [None] * NT

        def w2_a(i):  # attn transposes (bf16) + attnT eviction
            ws = _window_start(i)
            kb0 = ws // P
            nkb = min(DC, i - kb0 + 1)
            pt = psum_t.tile([P, 512], BF16, tag="ptb", bufs=3, name=f"atp{i}")
            for kk in range(nkb):
                nc.tensor.transpose(
                    out=pt[:, kk * P : (kk + 1) * P],
                    in_=attns[i][:, kk * P : (kk + 1) * P],
                    identity=ident_bf[:],
                )
            attnT = p_at.tile([P, 512], BF16, tag="attnT", bufs=3, name=f"attnT{i}")
            nc.scalar.copy(out=attnT[:, : nkb * P], in_=pt[:, : nkb * P])
            attnTs[i] = attnT

        def w2_b(i):  # o matmuls + scale
            ws = _window_start(i)
            kb0 = ws // P
            nkb = min(DC, i - kb0 + 1)
            ps_o = psum.tile([P, 512], F32, tag="ps", name=f"pso{i}")
            for kk in range(nkb):
                nc.tensor.matmul(
                    ps_o[:],
                    attnTs[i][:, kk * P : (kk + 1) * P],
                    v_sb[:, kb0 + kk, :],
                    start=(kk == 0),
                    stop=(kk == nkb - 1),
                )
            o_ps[i] = ps_o

        def s4_ln(j):  # residual + LN1 (o is already fully projected via B)
            r1 = p_st4.tile([P, D], F32, tag="r1", name=f"r1_{j}")
            # fused softmax-normalize + residual: r1 = o_psum * recip + h
            nc.vector.scalar_tensor_tensor(
                out=r1[:], in0=o_ps[j][:], scalar=recips[j][:, 0:1],
                in1=h_sb[:, j, :],
                op0=mybir.AluOpType.mult, op1=mybir.AluOpType.add,
            )
            if bc_bc is not None:
                nc.vector.tensor_add(out=r1[:], in0=r1[:], in1=bc_bc[:])
            stats = small.tile([P, 6], F32, tag="stats")
            nc.vector.bn_stats(out=stats[:], in_=r1[:])
            mv = small.tile([P, 2], F32, tag="mv")
            nc.vector.bn_aggr(out=mv[:], in_=stats[:])
            stdt = small.tile([P, 1], F32, tag="stdt")
            nc.scalar.activation(
                out=stdt[:], in_=mv[:, 1:2], func=AF.Sqrt,
                bias=eps_t[:, 0:1], scale=1.0,
            )
            rstd = small.tile([P, 1], F32, tag="rstd")
            nc.vector.reciprocal(out=rstd[:], in_=stdt[:])
            nc.vector.tensor_scalar(
                out=h1_sb[:, j, :], in0=r1[:],
                scalar1=mv[:, 0:1], scalar2=rstd[:, 0:1],
                op0=mybir.AluOpType.subtract, op1=mybir.AluOpType.mult,
            )
            if g1_bc is not None:
                nc.vector.tensor_mul(out=h1_sb[:, j, :], in0=h1_sb[:, j, :], in1=g1_bc[:])
            if be1_bc is not None:
                nc.vector.tensor_add(out=h1_sb[:, j, :], in0=h1_sb[:, j, :], in1=be1_bc[:])

        def s4_trans(j):  # h1 transposes (bf16) + h1T eviction
            pt3 = psum_t.tile([P, 512], BF16, tag="ptb", bufs=3, name=f"h1p{j}")
            for m in range(DC):
                nc.tensor.transpose(
                    out=pt3[:, m * P : (m + 1) * P],
                    in_=h1_sb[:, j, m * P : (m + 1) * P],
                    identity=ident_bf[:],
                )
            nc.scalar.copy(out=h1T[:, :, j * P : (j + 1) * P], in_=pt3[:])

        def ffn1_group_def_marker(): pass

        def ffn1_group(n, t):
            ps = psum.tile([P, 512], F32, tag="ps", name=f"psf{n}_{t}")
            for ki in range(DC):
                nc.tensor.matmul(
                    ps[:],
                    w1_sb[:, ki, n * P : (n + 1) * P],
                    h1T[:, ki, t * 512 : (t + 1) * 512],
                    start=(ki == 0),
                    stop=(ki == DC - 1),
                )
            fslc = f1T[:, n, t * 512 : (t + 1) * 512]
            if b1_sb is not None:
                nc.scalar.activation(
                    out=fslc, in_=ps[:], func=AF.Relu,
                    bias=b1_sb[:, n : n + 1], scale=1.0,
                )
            elif n % 2 == 0:
                nc.vector.tensor_scalar_max(out=fslc, in0=ps[:], scalar1=0.0)
            else:
                nc.scalar.activation(
                    out=fslc, in_=ps[:], func=AF.Relu, bias=0.0, scale=1.0,
                )

        for k in range(NT + 3):
            if k < NT:
                w2_a(k)
            if 1 <= k < NT + 1:
                w2_b(k - 1)
            if 2 <= k < NT + 2:
                s4_ln(k - 2)
            if 3 <= k:
                s4_trans(k - 3)

        p_st4.release()
        p_at.release()
        p_v.release()
        p_woh.release()

        p_w12 = tc.alloc_tile_pool(name="w12", bufs=1, side="right")
        w2_sb = p_w12.tile([P, FC, D], BF16, tag="w2")
        nc.sync.dma_start(out=w2_sb[:], in_=w2_d[:].rearrange("(k p) o -> p k o", p=P))
        p_f1 = tc.alloc_tile_pool(name="f1p", bufs=1, side="right")
        f1T = p_f1.tile([P, FC, SEQ], BF16, tag="f1T")

        # ---------- stage 6: FFN down + residual + LN2 (pipelined) ----------
        def s6_main(j):
            ps = psum.tile([P, 512], F32, tag="ps", name=f"ps6_{j}")
            for n in range(FC):
                nc.tensor.matmul(
                    ps[:],
                    f1T[:, n, j * P : (j + 1) * P],
                    w2_sb[:, n, :],
                    start=(n == 0),
                    stop=(n == FC - 1),
                )
            r2 = p_f1.tile([P, D], F32, tag="r2", bufs=3, name=f"r2_{j}")
            nc.vector.tensor_add(out=r2[:], in0=h1_sb[:, j, :], in1=ps[:])
            if b2_bc is not None:
                nc.vector.tensor_add(out=r2[:], in0=r2[:], in1=b2_bc[:])
            stats = small.tile([P, 6], F32, tag="stats")
            nc.vector.bn_stats(out=stats[:], in_=r2[:])
            mv = small.tile([P, 2], F32, tag="mv")
            nc.vector.bn_aggr(out=mv[:], in_=stats[:])
            stdt = small.tile([P, 1], F32, tag="stdt")
            # stdt = sqrt((var+eps))/SX so the LN output comes out x SX
            nc.scalar.activation(
                out=stdt[:], in_=mv[:, 1:2], func=AF.Sqrt,
                bias=eps2_t[:, 0:1], scale=1.0 / (SX * SX),
            )
            rstd = small.tile([P, 1], F32, tag="rstd")
            nc.vector.reciprocal(out=rstd[:], in_=stdt[:])
            h2_t = p_f1.tile([P, D], BF16, tag="h2_t", bufs=3, name=f"h2t_{j}")
            nc.vector.tensor_scalar(
                out=h2_t[:], in0=r2[:],
                scalar1=mv[:, 0:1], scalar2=rstd[:, 0:1],
                op0=mybir.AluOpType.subtract, op1=mybir.AluOpType.mult,
            )
            if g2_bc is not None:
                nc.vector.tensor_mul(out=h2_t[:], in0=h2_t[:], in1=g2_bc[:])
            if be2_bc is not None:
                nc.vector.tensor_add(out=h2_t[:], in0=h2_t[:], in1=be2_bc[:])
            return h2_t

        h2ts = [None] * NT

        def s6_trans(j):
            pt = psum_t.tile([P, 512], BF16, tag="ptb", bufs=3, name=f"h2p{j}")
            for m in range(DC):
                nc.tensor.transpose(
                    out=pt[:, m * P : (m + 1) * P],
                    in_=h2ts[j][:, m * P : (m + 1) * P],
                    identity=ident_bf[:],
                )
            # fp8 split: hi = q8(h2T), lo = q8(h2T - hi)
            nc.scalar.copy(out=h2Th[j][:, :, :], in_=pt[:])
            nc.vector.tensor_sub(
                out=h2Tl[j][:, :, :], in0=pt[:], in1=h2Th[j][:, :, :]
            )

        # head chunks for vc=0,1 interleaved into stage-6 so PE fills LN waits
        whh_r = whh_d[:].rearrange("(k p) v -> p k v", p=P)
        whl_r = whl_d[:].rearrange("(k p) v -> p k v", p=P)

        def load_whv(vc, name):
            wh = whpool.tile([P, DC, 512], F8, tag="whv", name=f"{name}h")
            nc.sync.dma_start(out=wh[:], in_=whh_r[:, :, vc * 512 : (vc + 1) * 512])
            wl = whpool.tile([P, DC, 512], F8, tag="whv", name=f"{name}l")
            nc.sync.dma_start(out=wl[:], in_=whl_r[:, :, vc * 512 : (vc + 1) * 512])
            return wh, wl

        NWARM = 5  # head chunks interleaved into stage 6
        whvw = []
        otw = []
        for vc in range(NWARM):
            whvw.append(load_whv(vc, f"whv{vc}"))
            otw.append(opool.tile([P, NT, 512], BF16 if OUT_BF16 else F32,
                                  tag="ot", bufs=NWARM + 1, name=f"otile{vc}"))

        def head_j(whv, otile, j, toggle):
            wh, wl = whv
            ps = psum.tile([P, 512], F32, tag="ps", name=f"psh{toggle}_{j}")
            # 3-term error-corrected fp8, all DoubleRow (contract 256/instr):
            #   xh@wh + xl@wh + xh@wl
            terms = ((h2Th[j], wh), (h2Tl[j], wh), (h2Th[j], wl))
            nterm = len(terms)
            for t_i, (xt, wt) in enumerate(terms):
                for k2 in range(DC // 2):
                    nc.tensor.matmul(
                        ps[:],
                        xt[:, 2 * k2 : 2 * k2 + 2, :],
                        wt[:, 2 * k2 : 2 * k2 + 2, :],
                        start=(t_i == 0 and k2 == 0),
                        stop=(t_i == nterm - 1 and k2 == DC // 2 - 1),
                        perf_mode=DR,
                    )
            if bh_sb_for(toggle) is not None:
                sc = whpool.tile([P, 512], F32, tag="hsc", bufs=2, name=f"hsc{toggle}_{j}")
                nc.scalar.activation(
                    out=sc[:], in_=ps[:], func=AF.Identity, bias=0.0, scale=INV_SXSW,
                )
                nc.vector.tensor_add(out=otile[:, j, :], in0=sc[:], in1=bh_sb_for(toggle)[:])
            elif j % 2 == 0:
                nc.vector.tensor_scalar_mul(out=otile[:, j, :], in0=ps[:], scalar1=INV_SXSW)
            else:
                nc.scalar.activation(
                    out=otile[:, j, :], in_=ps[:], func=AF.Identity,
                    bias=0.0, scale=INV_SXSW,
                )

        _bh_tiles = {}

        def bh_sb_for(key):
            return _bh_tiles.get(key)

        if bh_d is not None:
            for vc in range(NWARM):
                bhv = whpool.tile([P, 512], F32, tag="bh", bufs=2, name=f"bh{vc}")
                nc.sync.dma_start(
                    out=bhv[:], in_=_bcast_ap(bh_d[vc * 512 : (vc + 1) * 512])
                )
                _bh_tiles[vc] = bhv

        for t in range(SEQ // 512):
            for n in range(FC):
                ffn1_group(n, t)
                if t == 1 and n % 2 == 1:
                    h2ts[n // 2] = s6_main(n // 2)

        for k in range(NT + NWARM + 1):
            if 4 <= k < NT:
                h2ts[k] = s6_main(k)
            if 1 <= k <= NT:
                s6_trans(k - 1)
            for w in range(NWARM):
                if 2 + w <= k <= NT + 1 + w:
                    head_j(whvw[w], otw[w], k - 2 - w, w)
        out_rr = out_d[:].rearrange("(j p) v -> p j v", p=P)
        for vc in range(NWARM):
            nc.sync.dma_start(
                out=out_rr[:, :, vc * 512 : (vc + 1) * 512], in_=otw[vc][:]
            )

        p_f1.release()
        p_w12.release()
        p_h1.release()

        # ---------- stage 7: vocab head (vc >= 2) ----------
        out_r = out_d[:].rearrange("(j p) v -> p j v", p=P)
        for vc in range(NWARM, NV):
            whv = load_whv(vc, f"whv{vc}")
            if bh_d is not None:
                bh_bc = whpool.tile([P, 512], F32, tag="bh", bufs=2, name=f"bh{vc}")
                nc.sync.dma_start(
                    out=bh_bc[:], in_=_bcast_ap(bh_d[vc * 512 : (vc + 1) * 512])
                )
                _bh_tiles[vc] = bh_bc
            otile = opool.tile([P, NT, 512], BF16 if OUT_BF16 else F32,
                               tag="ot", bufs=NWARM + 1)
            # split stores so the final drain is short (esp. the last chunk)
            nstore = 4 if vc == NV - 1 else 2
            per = NT // nstore
            for j in range(NT):
                head_j(whv, otile, j, vc)
                if (j + 1) % per == 0:
                    nc.sync.dma_start(
                        out=out_r[:, j + 1 - per : j + 1, vc * 512 : (vc + 1) * 512],
                        in_=otile[:, j + 1 - per : j + 1, :],
                    )

        whpool.release()
        p_h2T.release()
        opool.release()
        psum_t.release()
        psum.release()
        small.release()
        const.release()

    nc.finalize()
    return nc


_PROGRAM_CACHE: dict = {}


def _get_program(flags: dict) -> bass.Bass:
    key = tuple(sorted(flags.items()))
    if key not in _PROGRAM_CACHE:
        _PROGRAM_CACHE[key] = _build_program(flags)
    return _PROGRAM_CACHE[key]


def _prep(x, embed_tab, row_embed, col_embed, Wq, bq, Wk, bk, Wv, bv, Wo, bo,
          ln1_g, ln1_b, W1, b1, W2, b2, ln2_g, ln2_b, Wh, bh):
    """Shared host-side prep: flags, common input map, per-core x shards."""
    f32c = lambda a: np.ascontiguousarray(np.asarray(a, dtype=np.float32))
    x = np.asarray(x)
    B = x.shape[0]
    assert x.shape == (B, SEQ)

    import ml_dtypes
    bfc = lambda a: np.ascontiguousarray(np.asarray(a, dtype=np.float32).astype(ml_dtypes.bfloat16))
    # fused attention weights (f32 host matmuls):
    #   scores = q k^T / sqrt(D) = h (Wq Wk^T / sqrt(D)) h^T   (biases zero)
    #   o = attn @ v @ Wo = attn @ (h (Wv Wo)) + (bv Wo + bo)
    wa = (f32c(Wq) @ f32c(Wk).T) * np.float32(1.0 / math.sqrt(D))
    wb = f32c(Wv) @ f32c(Wo)
    bc = f32c(bv) @ f32c(Wo) + f32c(bo)
    arrs = dict(
        wa=bfc(wa), wb=bfc(wb),
        w1=bfc(W1), w2=bfc(W2),
    )
    whs = f32c(Wh) * np.float32(SW)
    whh = whs.astype(ml_dtypes.float8_e4m3)
    whl = (whs - whh.astype(np.float32)).astype(ml_dtypes.float8_e4m3)
    arrs["whh"] = np.ascontiguousarray(whh)
    arrs["whl"] = np.ascontiguousarray(whl)
    pos = np.concatenate(
        [np.repeat(f32c(row_embed), GW, axis=0), np.tile(f32c(col_embed), (GH, 1))],
        axis=-1,
    ).astype(np.float32)
    arrs["maskt"] = _mask_tiles().astype(ml_dtypes.float8_e4m3)

    bias_map = dict(
        bc=bc, b1=f32c(b1),
        b2=f32c(b2), bh=f32c(bh),
        be1=f32c(ln1_b),
        # LN2's output is produced pre-scaled by SX; its bias must match
        be2=f32c(ln2_b) * np.float32(SX),
    )
    gain_map = dict(g1=f32c(ln1_g), g2=f32c(ln2_g))
    flags = {k: bool(np.any(v)) for k, v in bias_map.items()}
    flags.update({k: bool(np.any(v != 1.0)) for k, v in gain_map.items()})
    # the QK fusion drops per-row-constant score terms; valid only with
    # zero q/k biases (softmax shift-invariance covers the row-constant part)
    flags["bq"] = bool(np.any(f32c(bq)))
    flags["bk"] = bool(np.any(f32c(bk)))
    for k, v in {**bias_map, **gain_map}.items():
        if flags[k]:
            arrs[k] = v

    # host-side embedding gather + positional add, shipped per core in both
    # layouts: h [P, NT, D] token-major f32, ht [P, DC, SEQ] d-major bf16
    emb = f32c(embed_tab)
    hs, hts = [], []
    for c in range(B):
        h = emb[x[c]] + pos  # [SEQ, D] f32
        hs.append(np.ascontiguousarray(h.reshape(NT, P, D).transpose(1, 0, 2)))
        hts.append(np.ascontiguousarray(
            h.T.reshape(DC, P, SEQ).transpose(1, 0, 2).astype(ml_dtypes.bfloat16)
        ))
    return flags, arrs, hs, hts, B


def kernel(**inputs):
    flags, arrs, hs, hts, B = _prep(**inputs)
    nc = _get_program(flags)
    core_ids = list(range(8))
    in_maps = [{**arrs, "h": hs[c % B], "ht": hts[c % B]} for c in core_ids]
    res = run_bass_kernel_spmd(nc, in_maps, core_ids)
    out = np.stack([res.results[c]["out"] for c in range(B)], axis=0)
    return np.asarray(out, dtype=np.float32)



# revision 89
# speedup vs baseline: 1.3751x; 1.0026x over previous
"""Trainium2 Bass kernel for a small autoregressive transformer block with
local-windowed causal attention and a large (16k) vocab head.

Data-parallel over batch: batch item b runs on NeuronCore b (8 cores).
Per core (host precomputes h = emb[x]+pos, A = WqWk^T/sqrt(D), B = WvWo):
  s    = (h A) h^T + mask      banded (window <= 298), bf16 matmuls
  o    = softmax(s) @ (h B)    bf16; softmax-normalize fused into residual
  h1   = LN(h + o);  f = relu(h1@W1)@W2;  h2 = LN(h1 + f)   (bf16 matmuls)
  out  = h2 @ Wh               fp8-e4m3 DoubleRow with first-order error
                               correction: xh@wh + xl@wh + xh@wl, where
                               x = SX*h2 (folded into LN2), w = SW*Wh (host)

The vocab head is ~70%% of PE time; error-corrected fp8 DoubleRow runs it at
1.33x the bf16 rate with bf16-level accuracy (rel err ~4e-3 vs 2e-2 gate).
kernel(**inputs) takes full unsharded inputs, returns [8, 1024, 16384] f32.
"""

import math
import numpy as np

import concourse.bass as bass
import concourse.mybir as mybir
import concourse.tile as tile
from concourse import bacc
from concourse.bass_utils import run_bass_kernel_spmd
from concourse.masks import make_identity

# ---- problem constants (hardcoded per contract) ----
GH = 32
GW = 32
SEQ = 1024
WIN = 9
D = 512
DFF = 1024
VOCAB = 16384
EPS = 1e-5
NEG = -240.0  # exact in fp8-e4m3; exp(s + NEG) underflows to 0

P = 128
NT = SEQ // P        # 8 token chunks
DC = D // P          # 4 d chunks
FC = DFF // P        # 8 dff chunks
NV = VOCAB // 512    # 32 vocab chunks

F32 = mybir.dt.float32
BF16 = mybir.dt.bfloat16
F8 = mybir.dt.float8e4
OUT_BF16 = True
AF = mybir.ActivationFunctionType
DR = mybir.MatmulPerfMode.DoubleRow

# error-corrected fp8 head: logits = xh@wh + xl@wh + xh@wl, DoubleRow matmuls.
# h2 is produced pre-scaled by SX (folded into LN2's rsqrt); Wh is pre-scaled
# by SW on the host; the eviction copy divides by SX*SW.
SX = 8.0
SW = 32.0
INV_SXSW = 1.0 / (SX * SW)


def _window_start(i: int) -> int:
    # k-window [ws, ws+512) covers all allowed keys for query chunk i
    # (max lookback is WIN*GW + WIN = 297 < 384).
    return 128 * max(0, i - 3)


def _mask_tiles() -> np.ndarray:
    idx = np.arange(SEQ)
    r, c = idx // GW, idx % GW
    allow = (
        (np.abs(r[:, None] - r[None, :]) <= WIN)
        & (np.abs(c[:, None] - c[None, :]) <= WIN)
        & (idx[None, :] <= idx[:, None])
    )
    maskf = np.where(allow, 0.0, NEG).astype(np.float32)
    tiles = np.empty((NT, P, 512), np.float32)
    for i in range(NT):
        ws = _window_start(i)
        tiles[i] = maskf[i * P : (i + 1) * P, ws : ws + 512]
    return tiles


def _bcast_ap(a: bass.AP) -> bass.AP:
    """[n] DRAM vector AP -> [P, n] partition-broadcast DMA source."""
    return bass.AP(tensor=a.tensor, offset=a.offset, ap=[[0, P], *a.ap])


def _build_program(flags: dict, wh_bufs: int = 16) -> bass.Bass:
    nc = bacc.Bacc("TRN2", target_bir_lowering=False)

    # ---------- I/O ----------
    # h = emb[x] + pos is gathered host-side and shipped in both layouts:
    # token-major (residual adds, f32) and d-major/transposed (matmul
    # operand, bf16 — quantization is well under the error budget)
    h_d = nc.declare_dram_parameter("h", [P, NT, D], F32, False)
    ht_d = nc.declare_dram_parameter("ht", [P, DC, SEQ], BF16, False)
    msk_d = nc.declare_dram_parameter("maskt", [NT, P, 512], F8, False)
    # fused attention weights: A = Wq @ Wk^T / sqrt(D)  (scores = hA h^T),
    # B = Wv @ Wo  (o = attn @ (h B)); both computed host-side, shipped bf16.
    # wa is shipped m-major [P, m, ki, col] so per-column-block DMAs
    # have 1KB-contiguous descriptors
    wa_d = nc.declare_dram_parameter("wa", [P, DC, DC, P], BF16, False)
    wb_d = nc.declare_dram_parameter("wb", [D, D], BF16, False)
    w1_d = nc.declare_dram_parameter("w1", [D, DFF], BF16, False)
    w2_d = nc.declare_dram_parameter("w2", [DFF, D], BF16, False)
    whh_d = nc.declare_dram_parameter("whh", [D, VOCAB], F8, False)
    whl_d = nc.declare_dram_parameter("whl", [D, VOCAB], F8, False)
    dp = lambda name, shape: nc.declare_dram_parameter(name, shape, F32, False)
    assert not (flags["bq"] or flags["bk"]), (
        "QK-fused path requires zero q/k biases (true for this problem)"
    )
    # bc = bv @ Wo + bo, folded into the attention-output residual add
    bc_d = dp("bc", [D]) if flags["bc"] else None
    b1_d = dp("b1", [DFF]) if flags["b1"] else None
    b2_d = dp("b2", [D]) if flags["b2"] else None
    bh_d = dp("bh", [VOCAB]) if flags["bh"] else None
    g1_d = dp("g1", [D]) if flags["g1"] else None
    be1_d = dp("be1", [D]) if flags["be1"] else None
    g2_d = dp("g2", [D]) if flags["g2"] else None
    be2_d = dp("be2", [D]) if flags["be2"] else None
    out_d = nc.declare_dram_parameter("out", [SEQ, VOCAB], BF16 if OUT_BF16 else F32, True)

    with tile.TileContext(nc) as tc:
        # ----- whole-kernel pools -----
        const = tc.alloc_tile_pool(name="const", bufs=1)
        small = tc.alloc_tile_pool(name="small", bufs=8)
        psum = tc.alloc_tile_pool(name="psA", bufs=5, space="PSUM")
        psum_t = tc.alloc_tile_pool(name="psT", bufs=2, space="PSUM")
        opool = tc.alloc_tile_pool(name="outev", bufs=2, side="right")
        p_h2T = tc.alloc_tile_pool(name="h2Tp", bufs=1, side="right")

        ident_f = const.tile([P, P], F32, tag="ident_f")
        eps_t = const.tile([P, 1], F32, tag="eps")
        nc.vector.memset(eps_t[:], EPS)
        # eps for LN2 with the SX scale folded in: sqrt((var+eps)/SX^2)
        eps2_t = const.tile([P, 1], F32, tag="eps2")
        nc.vector.memset(eps2_t[:], EPS / (SX * SX))

        def load_col_bias(handle, nchunks, tag):
            # [nchunks*P] DRAM -> [P, nchunks] (chunk m in column m)
            t = const.tile([P, nchunks], F32, tag=tag)
            nc.sync.dma_start(out=t[:], in_=handle[:].rearrange("(m p) -> p m", p=P))
            return t

        def load_bcast(handle, n, tag):
            t = const.tile([P, n], F32, tag=tag)
            nc.sync.dma_start(out=t[:], in_=_bcast_ap(handle[:]))
            return t

        b1_sb = load_col_bias(b1_d, FC, "b1") if b1_d else None
        bc_bc = load_bcast(bc_d, D, "bc") if bc_d else None
        b2_bc = load_bcast(b2_d, D, "b2") if b2_d else None
        g1_bc = load_bcast(g1_d, D, "g1") if g1_d else None
        be1_bc = load_bcast(be1_d, D, "be1") if be1_d else None
        g2_bc = load_bcast(g2_d, D, "g2") if g2_d else None
        be2_bc = load_bcast(be2_d, D, "be2") if be2_d else None

        h2Th = [p_h2T.tile([P, DC, P], F8, tag=f"h2Th{j}", name=f"h2Th{j}") for j in range(NT)]
        h2Tl = [p_h2T.tile([P, DC, P], F8, tag=f"h2Tl{j}", name=f"h2Tl{j}") for j in range(NT)]

        # ----- phase A pools (left, LIFO) -----
        p_woh = tc.alloc_tile_pool(name="woh", bufs=1)         # h  (-> stage 4)
        h_sb = p_woh.tile([P, NT, D], F32, tag="h")

        p_v = tc.alloc_tile_pool(name="vp", bufs=1)            # v (-> wave 2)
        v_sb = p_v.tile([P, NT, D], BF16, tag="v")
        p_at = tc.alloc_tile_pool(name="attnw", bufs=3)        # softmax work (-> stage 4)
        p_qk = tc.alloc_tile_pool(name="qkp", bufs=1)          # qAT,hT (-> wave 1)
        qT = p_qk.tile([P, DC, SEQ], BF16, tag="qT")
        hT = p_qk.tile([P, DC, SEQ], BF16, tag="hT")

        p_wq = tc.alloc_tile_pool(name="wqp", bufs=1)          # wa,wb (-> stage 2)
        wa_sb = p_wq.tile([P, DC, DC, P], BF16, tag="wa")
        wb_sb = p_wq.tile([P, DC, D], BF16, tag="wb")

        # ---------- stage 1: load h (host-gathered emb[x]+pos) in both layouts
        # DMA order = stage-2 dependency order: wa col-chunk 0, hT half 0 by
        # ki (first matmul only needs ki=0), rest of wa, hT half 1, wb
        nc.sync.dma_start(out=wa_sb[:, 0, :, :], in_=wa_d[:, 0, :, :])
        for ki in range(DC):
            nc.sync.dma_start(out=hT[:, ki, 0:512], in_=ht_d[:, ki, 0:512])
        nc.sync.dma_start(out=wa_sb[:, 1, :, :], in_=wa_d[:, 1, :, :])
        # wb before hT half 1: the first v groups only touch hT[:, :, 0:512]
        nc.sync.dma_start(out=wb_sb[:], in_=wb_d[:].rearrange("(k p) o -> p k o", p=P))
        nc.sync.dma_start(out=wa_sb[:, 2, :, :], in_=wa_d[:, 2, :, :])
        nc.sync.dma_start(out=wa_sb[:, 3, :, :], in_=wa_d[:, 3, :, :])
        for ki in range(DC):
            nc.sync.dma_start(out=hT[:, ki, 512:1024], in_=ht_d[:, ki, 512:1024])

        make_identity(nc, ident_f[:])
        ident_bf = const.tile([P, P], BF16, tag="ident_bf")
        nc.vector.tensor_copy(out=ident_bf[:], in_=ident_f[:])
        ident_f8 = const.tile([P, P], F8, tag="ident_f8")
        nc.vector.tensor_copy(out=ident_f8[:], in_=ident_f[:])

        # ---------- stage 2 + wave 1 merged: qAT, scores/softmax, v ----------
        # all mask loads up front, then the deferred token-major h load
        msk_ts = []
        for i in range(NT):
            msk_t = p_at.tile([P, 512], F8, tag="msk", bufs=NT, name=f"msk{i}")
            nc.sync.dma_start(out=msk_t[:], in_=msk_d[i])
            msk_ts.append(msk_t)
        nc.sync.dma_start(out=h_sb[:], in_=h_d[:])

        attns = [None] * NT
        recips = [None] * NT

        def scores_i(i):
            ws = _window_start(i)
            nw = min(512, (i + 1) * P)  # live window (bf16: any size ok)
            ps_s = psum.tile([P, 512], F32, tag="ps")
            for ki in range(DC):
                nc.tensor.matmul(
                    ps_s[:, :nw],
                    qT[:, ki, i * P : (i + 1) * P],
                    hT[:, ki, ws : ws + nw],
                    start=(ki == 0),
                    stop=False,
                )
            # additive mask folded into the PSUM group: ps += I^T @ mask
            nc.tensor.matmul(
                ps_s[:, :nw], ident_f8[:], msk_ts[i][:, :nw], start=False, stop=True,
            )
            attn = p_at.tile([P, 512], BF16, tag="attn", bufs=NT, name=f"attn{i}")
            denom = small.tile([P, 1], F32, tag="denom")
            # A carries the 1/sqrt(D) factor, so the scores arrive pre-scaled
            nc.scalar.activation(
                out=attn[:, :nw], in_=ps_s[:, :nw], func=AF.Exp,
                bias=0.0, scale=1.0,
                accum_out=denom[:, 0:1],
            )
            recip = small.tile([P, 1], F32, tag="recip", bufs=NT, name=f"recip{i}")
            nc.vector.reciprocal(out=recip[:], in_=denom[:])
            attns[i] = attn
            recips[i] = recip

        # t-major order: all groups needing hT[0:512] first (PE is in-order);
        # scores for the finished t-half run among the v groups so the
        # softmax chain (ACT/DVE) hides behind stage-2 PE work
        for t in range(SEQ // 512):
            for m in range(DC):
                ps = psum.tile([P, 512], F32, tag="ps")
                for ki in range(DC):
                    nc.tensor.matmul(
                        ps[:],
                        wa_sb[:, m, ki, :],
                        hT[:, ki, t * 512 : (t + 1) * 512],
                        start=(ki == 0),
                        stop=(ki == DC - 1),
                    )
                dslc = qT[:, m, t * 512 : (t + 1) * 512]
                if m % 2 == 0:
                    nc.vector.tensor_copy(out=dslc, in_=ps[:])
                else:
                    nc.scalar.copy(out=dslc, in_=ps[:])
            for j in range(4 * t, 4 * t + 4):
                scores_i(j)
                ps = psum.tile([P, 512], F32, tag="ps")
                for ki in range(DC):
                    nc.tensor.matmul(
                        ps[:],
                        hT[:, ki, j * P : (j + 1) * P],
                        wb_sb[:, ki, :],
                        start=(ki == 0),
                        stop=(ki == DC - 1),
                    )
                if j % 2 == 0:
                    nc.scalar.copy(out=v_sb[:, j, :], in_=ps[:])
                else:
                    nc.vector.tensor_copy(out=v_sb[:, j, :], in_=ps[:])

        p_wq.release()
        p_qk.release()

        # ----- right-side pools for FFN phase -----
        whpool = tc.alloc_tile_pool(name="whstream", bufs=wh_bufs, side="right")
        p_h1 = tc.alloc_tile_pool(name="h1p", bufs=1, side="right")
        h1_sb = p_h1.tile([P, NT, D], BF16, tag="h1")
        h1T = p_h1.tile([P, DC, SEQ], BF16, tag="h1T")
        w1_sb = p_h1.tile([P, DC, DFF], BF16, tag="w1")
        nc.sync.dma_start(out=w1_sb[:], in_=w1_d[:].rearrange("(k p) o -> p k o", p=P))

        # ---------- stage 3 wave 2 + stage 4, software-pipelined ----------
        p_st4 = tc.alloc_tile_pool(name="st4", bufs=3)
        attnTs = [None] * NT
        o_ps = [None] * NT

        def w2_a(i):  # attn transposes (bf16) + attnT eviction
            ws = _window_start(i)
            kb0 = ws // P
            nkb = min(DC, i - kb0 + 1)
            pt = psum_t.tile([P, 512], BF16, tag="ptb", bufs=3, name=f"atp{i}")
            for kk in range(nkb):
                nc.tensor.transpose(
                    out=pt[:, kk * P : (kk + 1) * P],
                    in_=attns[i][:, kk * P : (kk + 1) * P],
                    identity=ident_bf[:],
                )
            attnT = p_at.tile([P, 512], BF16, tag="attnT", bufs=3, name=f"attnT{i}")
            nc.scalar.copy(out=attnT[:, : nkb * P], in_=pt[:, : nkb * P])
            attnTs[i] = attnT

        def w2_b(i):  # o matmuls + scale
            ws = _window_start(i)
            kb0 = ws // P
            nkb = min(DC, i - kb0 + 1)
            ps_o = psum.tile([P, 512], F32, tag="ps", name=f"pso{i}")
            for kk in range(nkb):
                nc.tensor.matmul(
                    ps_o[:],
                    attnTs[i][:, kk * P : (kk + 1) * P],
                    v_sb[:, kb0 + kk, :],
                    start=(kk == 0),
                    stop=(kk == nkb - 1),
                )
            o_ps[i] = ps_o

        def s4_ln(j):  # residual + LN1 (o is already fully projected via B)
            r1 = p_st4.tile([P, D], F32, tag="r1", name=f"r1_{j}")
            # fused softmax-normalize + residual: r1 = o_psum * recip + h
            nc.vector.scalar_tensor_tensor(
                out=r1[:], in0=o_ps[j][:], scalar=recips[j][:, 0:1],
                in1=h_sb[:, j, :],
                op0=mybir.AluOpType.mult, op1=mybir.AluOpType.add,
            )
            if bc_bc is not None:
                nc.vector.tensor_add(out=r1[:], in0=r1[:], in1=bc_bc[:])
            stats = small.tile([P, 6], F32, tag="stats")
            nc.vector.bn_stats(out=stats[:], in_=r1[:])
            mv = small.tile([P, 2], F32, tag="mv")
            nc.vector.bn_aggr(out=mv[:], in_=stats[:])
            stdt = small.tile([P, 1], F32, tag="stdt")
            nc.scalar.activation(
                out=stdt[:], in_=mv[:, 1:2], func=AF.Sqrt,
                bias=eps_t[:, 0:1], scale=1.0,
            )
            rstd = small.tile([P, 1], F32, tag="rstd")
            nc.vector.reciprocal(out=rstd[:], in_=stdt[:])
            nc.vector.tensor_scalar(
                out=h1_sb[:, j, :], in0=r1[:],
                scalar1=mv[:, 0:1], scalar2=rstd[:, 0:1],
                op0=mybir.AluOpType.subtract, op1=mybir.AluOpType.mult,
            )
            if g1_bc is not None:
                nc.vector.tensor_mul(out=h1_sb[:, j, :], in0=h1_sb[:, j, :], in1=g1_bc[:])
            if be1_bc is not None:
                nc.vector.tensor_add(out=h1_sb[:, j, :], in0=h1_sb[:, j, :], in1=be1_bc[:])

        def s4_trans(j):  # h1 transposes (bf16) + h1T eviction
            pt3 = psum_t.tile([P, 512], BF16, tag="ptb", bufs=3, name=f"h1p{j}")
            for m in range(DC):
                nc.tensor.transpose(
                    out=pt3[:, m * P : (m + 1) * P],
                    in_=h1_sb[:, j, m * P : (m + 1) * P],
                    identity=ident_bf[:],
                )
            nc.scalar.copy(out=h1T[:, :, j * P : (j + 1) * P], in_=pt3[:])

        def ffn1_group_def_marker(): pass

        def ffn1_group(n, t):
            ps = psum.tile([P, 512], F32, tag="ps", name=f"psf{n}_{t}")
            for ki in range(DC):
                nc.tensor.matmul(
                    ps[:],
                    w1_sb[:, ki, n * P : (n + 1) * P],
                    h1T[:, ki, t * 512 : (t + 1) * 512],
                    start=(ki == 0),
                    stop=(ki == DC - 1),
                )
            fslc = f1T[:, n, t * 512 : (t + 1) * 512]
            if b1_sb is not None:
                nc.scalar.activation(
                    out=fslc, in_=ps[:], func=AF.Relu,
                    bias=b1_sb[:, n : n + 1], scale=1.0,
                )
            elif n % 2 == 0:
                nc.vector.tensor_scalar_max(out=fslc, in0=ps[:], scalar1=0.0)
            else:
                nc.scalar.activation(
                    out=fslc, in_=ps[:], func=AF.Relu, bias=0.0, scale=1.0,
                )

        for k in range(NT + 3):
            if k < NT:
                w2_a(k)
            if 1 <= k < NT + 1:
                w2_b(k - 1)
            if 2 <= k < NT + 2:
                s4_ln(k - 2)
            if 3 <= k:
                s4_trans(k - 3)

        p_st4.release()
        p_at.release()
        p_v.release()
        p_woh.release()

        p_w12 = tc.alloc_tile_pool(name="w12", bufs=1, side="right")
        w2_sb = p_w12.tile([P, FC, D], BF16, tag="w2")
        nc.sync.dma_start(out=w2_sb[:], in_=w2_d[:].rearrange("(k p) o -> p k o", p=P))
        p_f1 = tc.alloc_tile_pool(name="f1p", bufs=1, side="right")
        f1T = p_f1.tile([P, FC, SEQ], BF16, tag="f1T")

        # ---------- stage 6: FFN down + residual + LN2 (pipelined) ----------
        def s6_main(j):
            ps = psum.tile([P, 512], F32, tag="ps", name=f"ps6_{j}")
            for n in range(FC):
                nc.tensor.matmul(
                    ps[:],
                    f1T[:, n, j * P : (j + 1) * P],
                    w2_sb[:, n, :],
                    start=(n == 0),
                    stop=(n == FC - 1),
                )
            r2 = p_f1.tile([P, D], F32, tag="r2", bufs=3, name=f"r2_{j}")
            nc.vector.tensor_add(out=r2[:], in0=h1_sb[:, j, :], in1=ps[:])
            if b2_bc is not None:
                nc.vector.tensor_add(out=r2[:], in0=r2[:], in1=b2_bc[:])
            stats = small.tile([P, 6], F32, tag="stats")
            nc.vector.bn_stats(out=stats[:], in_=r2[:])
            mv = small.tile([P, 2], F32, tag="mv")
            nc.vector.bn_aggr(out=mv[:], in_=stats[:])
            stdt = small.tile([P, 1], F32, tag="stdt")
            # stdt = sqrt((var+eps))/SX so the LN output comes out x SX
            nc.scalar.activation(
                out=stdt[:], in_=mv[:, 1:2], func=AF.Sqrt,
                bias=eps2_t[:, 0:1], scale=1.0 / (SX * SX),
            )
            rstd = small.tile([P, 1], F32, tag="rstd")
            nc.vector.reciprocal(out=rstd[:], in_=stdt[:])
            h2_t = p_f1.tile([P, D], BF16, tag="h2_t", bufs=3, name=f"h2t_{j}")
            nc.vector.tensor_scalar(
                out=h2_t[:], in0=r2[:],
                scalar1=mv[:, 0:1], scalar2=rstd[:, 0:1],
                op0=mybir.AluOpType.subtract, op1=mybir.AluOpType.mult,
            )
            if g2_bc is not None:
                nc.vector.tensor_mul(out=h2_t[:], in0=h2_t[:], in1=g2_bc[:])
            if be2_bc is not None:
                nc.vector.tensor_add(out=h2_t[:], in0=h2_t[:], in1=be2_bc[:])
            return h2_t

        h2ts = [None] * NT

        def s6_trans(j):
            pt = psum_t.tile([P, 512], BF16, tag="ptb", bufs=3, name=f"h2p{j}")
            for m in range(DC):
                nc.tensor.transpose(
                    out=pt[:, m * P : (m + 1) * P],
                    in_=h2ts[j][:, m * P : (m + 1) * P],
                    identity=ident_bf[:],
                )
            # fp8 split: hi = q8(h2T), lo = q8(h2T - hi)
            nc.scalar.copy(out=h2Th[j][:, :, :], in_=pt[:])
            nc.vector.tensor_sub(
                out=h2Tl[j][:, :, :], in0=pt[:], in1=h2Th[j][:, :, :]
            )

        # head chunks for vc=0,1 interleaved into stage-6 so PE fills LN waits
        whh_r = whh_d[:].rearrange("(k p) v -> p k v", p=P)
        whl_r = whl_d[:].rearrange("(k p) v -> p k v", p=P)

        def load_whv(vc, name):
            wh = whpool.tile([P, DC, 512], F8, tag="whv", name=f"{name}h")
            nc.sync.dma_start(out=wh[:], in_=whh_r[:, :, vc * 512 : (vc + 1) * 512])
            wl = whpool.tile([P, DC, 512], F8, tag="whv", name=f"{name}l")
            nc.sync.dma_start(out=wl[:], in_=whl_r[:, :, vc * 512 : (vc + 1) * 512])
            return wh, wl

        NWARM = 5  # head chunks interleaved into stage 6
        whvw = []
        otw = []
        for vc in range(NWARM):
            whvw.append(load_whv(vc, f"whv{vc}"))
            otw.append(opool.tile([P, NT, 512], BF16 if OUT_BF16 else F32,
                                  tag="ot", bufs=NWARM + 1, name=f"otile{vc}"))

        def head_j(whv, otile, j, toggle):
            wh, wl = whv
            ps = psum.tile([P, 512], F32, tag="ps", name=f"psh{toggle}_{j}")
            # 3-term error-corrected fp8, all DoubleRow (contract 256/instr):
            #   xh@wh + xl@wh + xh@wl
            terms = ((h2Th[j], wh), (h2Tl[j], wh), (h2Th[j], wl))
            nterm = len(terms)
            for t_i, (xt, wt) in enumerate(terms):
                for k2 in range(DC // 2):
                    nc.tensor.matmul(
                        ps[:],
                        xt[:, 2 * k2 : 2 * k2 + 2, :],
                        wt[:, 2 * k2 : 2 * k2 + 2, :],
                        start=(t_i == 0 and k2 == 0),
                        stop=(t_i == nterm - 1 and k2 == DC // 2 - 1),
                        perf_mode=DR,
                    )
            if bh_sb_for(toggle) is not None:
                sc = whpool.tile([P, 512], F32, tag="hsc", bufs=2, name=f"hsc{toggle}_{j}")
                nc.scalar.activation(
                    out=sc[:], in_=ps[:], func=AF.Identity, bias=0.0, scale=INV_SXSW,
                )
                nc.vector.tensor_add(out=otile[:, j, :], in0=sc[:], in1=bh_sb_for(toggle)[:])
            elif j % 2 == 0:
                nc.vector.tensor_scalar_mul(out=otile[:, j, :], in0=ps[:], scalar1=INV_SXSW)
            else:
                nc.scalar.activation(
                    out=otile[:, j, :], in_=ps[:], func=AF.Identity,
                    bias=0.0, scale=INV_SXSW,
                )

        _bh_tiles = {}

        def bh_sb_for(key):
            return _bh_tiles.get(key)

        if bh_d is not None:
            for vc in range(NWARM):
                bhv = whpool.tile([P, 512], F32, tag="bh", bufs=2, name=f"bh{vc}")
                nc.sync.dma_start(
                    out=bhv[:], in_=_bcast_ap(bh_d[vc * 512 : (vc + 1) * 512])
                )
                _bh_tiles[vc] = bhv

        for t in range(SEQ // 512):
            for n in range(FC):
                ffn1_group(n, t)
                if t == 1 and n % 2 == 1:
                    h2ts[n // 2] = s6_main(n // 2)

        for k in range(NT + NWARM + 1):
            if 4 <= k < NT:
                h2ts[k] = s6_main(k)
            if 1 <= k <= NT:
                s6_trans(k - 1)
            for w in range(NWARM):
                if 2 + w <= k <= NT + 1 + w:
                    head_j(whvw[w], otw[w], k - 2 - w, w)
        out_rr = out_d[:].rearrange("(j p) v -> p j v", p=P)
        for vc in range(NWARM):
            nc.sync.dma_start(
                out=out_rr[:, :, vc * 512 : (vc + 1) * 512], in_=otw[vc][:]
            )

        p_f1.release()
        p_w12.release()
        p_h1.release()

        # ---------- stage 7: vocab head (vc >= 2) ----------
        out_r = out_d[:].rearrange("(j p) v -> p j v", p=P)
        for vc in range(NWARM, NV):
            whv = load_whv(vc, f"whv{vc}")
            if bh_d is not None:
                bh_bc = whpool.tile([P, 512], F32, tag="bh", bufs=2, name=f"bh{vc}")
                nc.sync.dma_start(
                    out=bh_bc[:], in_=_bcast_ap(bh_d[vc * 512 : (vc + 1) * 512])
                )
                _bh_tiles[vc] = bh_bc
            otile = opool.tile([P, NT, 512], BF16 if OUT_BF16 else F32,
                               tag="ot", bufs=NWARM + 1)
            # split stores so the final drain is short (esp. the last chunk)
            nstore = 4 if vc == NV - 1 else 2
            per = NT // nstore
            for j in range(NT):
                head_j(whv, otile, j, vc)
                if (j + 1) % per == 0:
                    nc.sync.dma_start(
                        out=out_r[:, j + 1 - per : j + 1, vc * 512 : (vc + 1) * 512],
                        in_=otile[:, j + 1 - per : j + 1, :],
                    )

        whpool.release()
        p_h2T.release()
        opool.release()
        psum_t.release()
        psum.release()
        small.release()
        const.release()

    nc.finalize()
    return nc


_PROGRAM_CACHE: dict = {}


def _get_program(flags: dict) -> bass.Bass:
    key = tuple(sorted(flags.items()))
    if key not in _PROGRAM_CACHE:
        _PROGRAM_CACHE[key] = _build_program(flags)
    return _PROGRAM_CACHE[key]


def _prep(x, embed_tab, row_embed, col_embed, Wq, bq, Wk, bk, Wv, bv, Wo, bo,
          ln1_g, ln1_b, W1, b1, W2, b2, ln2_g, ln2_b, Wh, bh):
    """Shared host-side prep: flags, common input map, per-core x shards."""
    f32c = lambda a: np.ascontiguousarray(np.asarray(a, dtype=np.float32))
    x = np.asarray(x)
    B = x.shape[0]
    assert x.shape == (B, SEQ)

    import ml_dtypes
    bfc = lambda a: np.ascontiguousarray(np.asarray(a, dtype=np.float32).astype(ml_dtypes.bfloat16))
    # fused attention weights (f32 host matmuls):
    #   scores = q k^T / sqrt(D) = h (Wq Wk^T / sqrt(D)) h^T   (biases zero)
    #   o = attn @ v @ Wo = attn @ (h (Wv Wo)) + (bv Wo + bo)
    wa = (f32c(Wq) @ f32c(Wk).T) * np.float32(1.0 / math.sqrt(D))
    wb = f32c(Wv) @ f32c(Wo)
    bc = f32c(bv) @ f32c(Wo) + f32c(bo)
    # wa m-major: wa_t[p, m, k, c] = wa[k*P+p, m*P+c]
    wa_t = wa.reshape(DC, P, DC, P).transpose(1, 2, 0, 3)
    arrs = dict(
        wa=bfc(wa_t), wb=bfc(wb),
        w1=bfc(W1), w2=bfc(W2),
    )
    whs = f32c(Wh) * np.float32(SW)
    whh = whs.astype(ml_dtypes.float8_e4m3)
    whl = (whs - whh.astype(np.float32)).astype(ml_dtypes.float8_e4m3)
    arrs["whh"] = np.ascontiguousarray(whh)
    arrs["whl"] = np.ascontiguousarray(whl)
    pos = np.concatenate(
        [np.repeat(f32c(row_embed), GW, axis=0), np.tile(f32c(col_embed), (GH, 1))],
        axis=-1,
    ).astype(np.float32)
    arrs["maskt"] = _mask_tiles().astype(ml_dtypes.float8_e4m3)

    bias_map = dict(
        bc=bc, b1=f32c(b1),
        b2=f32c(b2), bh=f32c(bh),
        be1=f32c(ln1_b),
        # LN2's output is produced pre-scaled by SX; its bias must match
        be2=f32c(ln2_b) * np.float32(SX),
    )
    gain_map = dict(g1=f32c(ln1_g), g2=f32c(ln2_g))
    flags = {k: bool(np.any(v)) for k, v in bias_map.items()}
    flags.update({k: bool(np.any(v != 1.0)) for k, v in gain_map.items()})
    # the QK fusion drops per-row-constant score terms; valid only with
    # zero q/k biases (softmax shift-invariance covers the row-constant part)
    flags["bq"] = bool(np.any(f32c(bq)))
    flags["bk"] = bool(np.any(f32c(bk)))
    for k, v in {**bias_map, **gain_map}.items():
        if flags[k]:
            arrs[k] = v

    # host-side embedding gather + positional add, shipped per core in both
    # layouts: h [P, NT, D] token-major f32, ht [P, DC, SEQ] d-major bf16
    emb = f32c(embed_tab)
    hs, hts = [], []
    for c in range(B):
        h = emb[x[c]] + pos  # [SEQ, D] f32
        hs.append(np.ascontiguousarray(h.reshape(NT, P, D).transpose(1, 0, 2)))
        hts.append(np.ascontiguousarray(
            h.T.reshape(DC, P, SEQ).transpose(1, 0, 2).astype(ml_dtypes.bfloat16)
        ))
    return flags, arrs, hs, hts, B


def kernel(**inputs):
    flags, arrs, hs, hts, B = _prep(**inputs)
    nc = _get_program(flags)
    core_ids = list(range(8))
    in_maps = [{**arrs, "h": hs[c % B], "ht": hts[c % B]} for c in core_ids]
    res = run_bass_kernel_spmd(nc, in_maps, core_ids)
    out = np.stack([res.results[c]["out"] for c in range(B)], axis=0)
    return np.asarray(out, dtype=np.float32)

